# revision 1
# baseline (speedup 1.0000x reference)
"""Multi-head attention (q/k/v projections + softmax attention + out-projection)
on 8 Trainium2 NeuronCores.

Sharding: 16 (batch, head) units over 8 cores -> core c handles batch n = c//4
and head pair hp = c%4 (columns 128*hp : 128*hp+128 of the projections).
Per-core partial outputs (each pair's contribution to mix @ Wo) are summed on
host per batch, + bo.

Device kernel (per core):
  - Host pre-transposes q[n],k[n],v[n] -> xT [512, 4096] so the D-contraction
    projections need no on-device transpose.
  - All matmuls run in float32r (TF32-like, 1 cycle/row vs 4 for fp32 on the
    PE; measured end-to-end |err|_max/|out|_max ~ 5e-4 vs fp32's 3e-6).
  - QPT/KPT [128, 512]x8 chunk tiles: W.T @ x with head-dim on partitions;
    q scaled by 1/8, biases folded in via DVE tensor_scalar(mult, add).
  - VP chunk tiles [128lkv, 4, 130]: v-projection computed un-transposed (lkv
    on partitions) with the bias added via an extra K=1 ones x bv matmul row;
    layout per j: [h0 c(64) | ones | h1 c(64) | ones] - the ones column makes
    each PV matmul also accumulate sum(exp) into psum row 64.
  - Attention in S^T orientation: S^T[lkv,lq] = KPT_h.T @ QPT_h (K=64, heads
    at PE row groups 0/64), exp on ScalarE with FD=1024 tiles (no max
    subtraction needed: scores ~ N(0,1), exp range is tiny), PV accumulates
    mixT[c,lq] + sumexp transpose-free. ScalarE exp (~270us busy) is the
    critical engine; S/PV matmuls (~220us) hide under it.
  - sumexp [1,1024] is transposed to [128,8] partition-major via a DRAM
    bounce, reciprocal on DVE, and the normalization is folded into the
    out-projection: per-head out-proj psums scaled per-partition (lq) by
    1/sumexp on DVE, then summed on GPSIMD.
  - One unified 8-bank PSUM pool (st0/st1/pv0/pv1 tags x 2 banks); the
    projection phase rotates through the same tags.
"""

import numpy as np

import concourse.bacc as bacc
import concourse.mybir as mybir
import concourse.tile as tile
from concourse import bass_utils

P = 128
L = 4096
D = 512
F32 = mybir.dt.float32
F32R_DT = mybir.dt.float32r
AF = mybir.ActivationFunctionType

_NC = None
F32R = True  # run matmuls in float32r (TF32-like, 4x faster than fp32 on PE)


def _mm(nc, out, lhsT, rhs, f32r=True, **kw):
    nc.tensor.matmul(out, lhsT=lhsT, rhs=rhs, **kw)


def build():
    nc = bacc.Bacc("TRN2", target_bir_lowering=False, debug=False)

    xqt = nc.dram_tensor("xqt", (D, L), F32R_DT, kind="ExternalInput").ap()
    xkt = nc.dram_tensor("xkt", (D, L), F32R_DT, kind="ExternalInput").ap()
    xvt = nc.dram_tensor("xvt", (D, L), F32R_DT, kind="ExternalInput").ap()
    wq = nc.dram_tensor("wq", (D, P), F32R_DT, kind="ExternalInput").ap()
    wk = nc.dram_tensor("wk", (D, P), F32R_DT, kind="ExternalInput").ap()
    wv = nc.dram_tensor("wv", (D, P), F32R_DT, kind="ExternalInput").ap()
    wo = nc.dram_tensor("wo", (P, D), F32R_DT, kind="ExternalInput").ap()
    bqs = nc.dram_tensor("bqs", (P, 1), F32, kind="ExternalInput").ap()
    bkc = nc.dram_tensor("bkc", (P, 1), F32, kind="ExternalInput").ap()
    bvr = nc.dram_tensor("bvr", (1, P), F32R_DT, kind="ExternalInput").ap()
    out = nc.dram_tensor("out", (L, D), F32, kind="ExternalOutput").ap()

    with tile.TileContext(nc) as tc:
        with tc.tile_pool(name="const", bufs=1) as const, \
             tc.tile_pool(name="persist", bufs=1) as persist:
            wq_sb = const.tile([P, 4, P], F32R_DT, tag="wq")
            nc.sync.dma_start(wq_sb, wq.rearrange("(o p) m -> p o m", p=P))
            wk_sb = const.tile([P, 4, P], F32R_DT, tag="wk")
            nc.sync.dma_start(wk_sb, wk.rearrange("(o p) m -> p o m", p=P))
            wv_sb = const.tile([P, 4, P], F32R_DT, tag="wv")
            nc.sync.dma_start(wv_sb, wv.rearrange("(o p) m -> p o m", p=P))
            wo_sb = const.tile([P, D], F32R_DT, tag="wo")
            nc.sync.dma_start(wo_sb, wo)
            bq_sb = const.tile([P, 1], F32, tag="bq")
            nc.sync.dma_start(bq_sb, bqs)
            bk_sb = const.tile([P, 1], F32, tag="bk")
            nc.sync.dma_start(bk_sb, bkc)
            bvr_sb = const.tile([1, P], F32R_DT, tag="bvr")
            nc.sync.dma_start(bvr_sb, bvr)
            onesr = const.tile([1, P], F32R_DT, tag="onesr")
            nc.scalar.activation(onesr, bvr_sb, AF.Identity,
                                 bias=1.0, scale=0.0)

            qpt_t = [persist.tile([P, 512], F32R_DT, tag=f"qpt{c}",
                                  name=f"qpt{c}") for c in range(8)]
            kpt_t = [persist.tile([P, 512], F32R_DT, tag=f"kpt{c}",
                                  name=f"kpt{c}") for c in range(8)]
            vp_t = [persist.tile([P, 4, 130], F32R_DT, tag=f"vp{c}",
                                 name=f"vp{c}") for c in range(8)]
            ones_in = bq_sb[:, :, None].to_broadcast((P, 4, 1))
            for c in range(8):
                nc.scalar.activation(vp_t[c][:, :, 64:65], ones_in,
                                     AF.Identity, bias=1.0, scale=0.0)
                nc.scalar.activation(vp_t[c][:, :, 129:130], ones_in,
                                     AF.Identity, bias=1.0, scale=0.0)

            # ---------------- projections + attention ----------------
            # One PSUM pool for everything (8 banks exactly): st0/st1 and
            # pv0/pv1 tags, [128,1024] fp32 = 2 banks each. Projections
            # rotate through the same tags so attention tiles never wait on
            # a disjoint pool's address range.
            with tc.tile_pool(name="xs", bufs=2) as xs, \
                 tc.tile_pool(name="psp", bufs=1, space="PSUM") as psp, \
                 tc.tile_pool(name="esp", bufs=4) as esp, \
                 tc.tile_pool(name="smallp", bufs=4) as smallp, \
                 tc.tile_pool(name="mixp", bufs=4) as mixp, \
                 tc.tile_pool(name="outp", bufs=3) as outp, \
                 tc.tile_pool(name="dramp", bufs=2, space="DRAM") as dramp:
                xqv = xqt.rearrange("(o p) l -> p o l", p=P)
                xkv = xkt.rearrange("(o p) l -> p o l", p=P)
                xvv = xvt.rearrange("(o p) l -> p o l", p=P)
                for ch in range(8):
                    sl = slice(ch * 512, (ch + 1) * 512)
                    # K chunk
                    xtk = xs.tile([P, 4, 512], F32R_DT, tag="xtk")
                    nc.sync.dma_start(xtk, xkv[:, :, sl])
                    ps = psp.tile([P, 512], F32, tag="st0", name="kps")
                    for dk in range(4):
                        _mm(nc, ps, wk_sb[:, dk, :], xtk[:, dk, :],
                            start=(dk == 0), stop=(dk == 3))
                    nc.vector.tensor_scalar(
                        kpt_t[ch][:], ps, 1.0, bk_sb,
                        mybir.AluOpType.mult, mybir.AluOpType.add)
                    # V chunk
                    xtv = xs.tile([P, 4, 512], F32R_DT, tag="xtv")
                    nc.gpsimd.dma_start(xtv, xvv[:, :, sl])
                    for js in range(4):
                        j = ch * 4 + js
                        psv = psp.tile([P, P], F32, tag=f"pv{js % 2}",
                                       name="psv")
                        for dk in range(4):
                            _mm(nc, psv, xtv[:, dk, js * P:(js + 1) * P],
                                wv_sb[:, dk, :],
                                start=(dk == 0), stop=False)
                        _mm(nc, psv, onesr, bvr_sb,
                            start=False, stop=True)
                        nc.vector.tensor_copy(vp_t[ch][:, js, 0:64],
                                               psv[:, 0:64])
                        nc.vector.tensor_copy(vp_t[ch][:, js, 65:129],
                                              psv[:, 64:128])
                    # Q chunk
                    xtq = xs.tile([P, 4, 512], F32R_DT, tag="xtq")
                    nc.gpsimd.dma_start(xtq, xqv[:, :, sl])
                    psq = psp.tile([P, 512], F32, tag="st1", name="qps")
                    for dk in range(4):
                        _mm(nc, psq, wq_sb[:, dk, :], xtq[:, dk, :],
                            start=(dk == 0), stop=(dk == 3))
                    nc.vector.tensor_scalar(
                        qpt_t[ch][:], psq, 0.125, bq_sb,
                        mybir.AluOpType.mult, mybir.AluOpType.add)

                # ---------------- attention ----------------
                for lqc in range(4):
                    q0 = lqc * 1024
                    pv_ps = [psp.tile([P, 1024], F32, tag=f"pv{h}", name=f"pv{h}")
                             for h in range(2)]
                    for j in range(32):
                        for h in range(2):
                            hb = h * 64
                            st = psp.tile([P, 1024], F32, tag=f"st{h}")
                            for hf in range(2):
                                _mm(nc, st[:, hf * 512:(hf + 1) * 512],
                                    kpt_t[j // 4][hb:hb + 64,
                                                  (j % 4) * P:(j % 4 + 1) * P],
                                    qpt_t[2 * lqc + hf][hb:hb + 64, :],
                                    start=True, stop=True)
                            est = esp.tile([P, 1024], F32R_DT, tag=f"est{h}")
                            nc.scalar.activation(est, st, AF.Exp)
                            for hf in range(2):
                                _mm(nc, pv_ps[h][0:65, hf * 512:(hf + 1) * 512],
                                    vp_t[j // 4][:, j % 4, h * 65:(h + 1) * 65],
                                    est[:, hf * 512:(hf + 1) * 512],
                                    start=(j == 0), stop=(j == 31))
                    recips = []
                    mix2 = mixp.tile([P, 1024], F32R_DT, tag="mix2")
                    for h in range(2):
                        row = smallp.tile([1, 1024], F32, tag=f"row{h}")
                        nc.vector.tensor_copy(row, pv_ps[h][64:65, :])
                        drow = dramp.tile([1024], F32, tag=f"drow{h}")
                        nc.sync.dma_start(drow, row)
                        sumsT = smallp.tile([P, 8], F32, tag=f"sT{h}")
                        # sumsT[p, s] = sums[s*128 + p]
                        nc.sync.dma_start(
                            sumsT, drow.rearrange("(s p) -> p s", p=P))
                        rT = smallp.tile([P, 8], F32, tag=f"rT{h}")
                        nc.vector.reciprocal(rT, sumsT)
                        recips.append(rT)
                        nc.vector.tensor_copy(mix2[h * 64:(h + 1) * 64, :],
                                              pv_ps[h][0:64, :])
                    for s in range(8):
                        ops = [psp.tile([P, D], F32, tag=f"pv{h}", name=f"op{h}")
                               for h in range(2)]
                        for h in range(2):
                            _mm(nc, ops[h],
                                mix2[h * 64:(h + 1) * 64, s * P:(s + 1) * P],
                                wo_sb[h * 64:(h + 1) * 64, :],
                                start=True, stop=True)
                        t0 = outp.tile([P, D], F32, tag="t0")
                        nc.vector.tensor_scalar_mul(t0, ops[0],
                                                    recips[0][:, s:s + 1])
                        t1 = outp.tile([P, D], F32, tag="t1")
                        nc.vector.tensor_scalar_mul(t1, ops[1],
                                                    recips[1][:, s:s + 1])
                        ob = outp.tile([P, D], F32, tag="ob")
                        nc.gpsimd.tensor_add(ob, t0, t1)
                        nc.sync.dma_start(
                            out[q0 + s * P:q0 + (s + 1) * P, :], ob)

    nc.compile()
    return nc


def get_nc():
    global _NC
    if _NC is None:
        _NC = build()
    return _NC


def make_in_maps(q, k, v, Wq, bq, Wk, bk, Wv, bv, Wo, bo):
    q = np.asarray(q, np.float32)
    k = np.asarray(k, np.float32)
    v = np.asarray(v, np.float32)
    Wq = np.asarray(Wq, np.float32)
    Wk = np.asarray(Wk, np.float32)
    Wv = np.asarray(Wv, np.float32)
    Wo = np.asarray(Wo, np.float32)
    bq = np.asarray(bq, np.float32)
    bk = np.asarray(bk, np.float32)
    bv = np.asarray(bv, np.float32)
    xts = {}
    for n in range(2):
        xts[n] = (np.ascontiguousarray(q[n].T),
                  np.ascontiguousarray(k[n].T),
                  np.ascontiguousarray(v[n].T))
    in_maps = []
    for c in range(8):
        n, hp = c // 4, c % 4
        sl = slice(P * hp, P * (hp + 1))
        xq, xk, xv = xts[n]
        in_maps.append({
            "xqt": xq, "xkt": xk, "xvt": xv,
            "wq": np.ascontiguousarray(Wq[:, sl]),
            "wk": np.ascontiguousarray(Wk[:, sl]),
            "wv": np.ascontiguousarray(Wv[:, sl]),
            "wo": np.ascontiguousarray(Wo[sl, :]),
            "bqs": (bq[sl] * 0.125).reshape(P, 1).astype(np.float32),
            "bkc": bk[sl].reshape(P, 1).astype(np.float32),
            "bvr": bv[sl].reshape(1, P).astype(np.float32),
        })
    return in_maps


def assemble(results, bo):
    bo = np.asarray(bo, np.float32)
    out = np.zeros((2, L, D), np.float32)
    for c in range(8):
        out[c // 4] += results[c]["out"]
    out += bo[None, None, :]
    return out


def kernel(q, k, v, Wq, bq, Wk, bk, Wv, bv, Wo, bo):
    nc = get_nc()
    in_maps = make_in_maps(q, k, v, Wq, bq, Wk, bk, Wv, bv, Wo, bo)
    res = bass_utils.run_bass_kernel_spmd(nc, in_maps, core_ids=list(range(8)))
    return assemble(res.results, bo)


if __name__ == "__main__":
    build()
    print("build ok")



# revision 17
# speedup vs baseline: 1.6395x; 1.6395x over previous
"""Multi-head attention (q/k/v projections + softmax attention + out-projection)
on 8 Trainium2 NeuronCores.

Sharding: 16 (batch, head) units over 8 cores -> core c handles batch n = c//4
and head pair hp = c%4 (channels 128*hp : 128*hp+128). Host sums the 4 partial
outputs per batch and adds bo.

v4 design (baseline was 418us):
  - fp16 operand universe: x, W, qpt/kpt [128ch, 4096] fp16, vp [128kv, 32,
    130] fp16 (cols 64/129 = ones for sumexp), Wo fp16, mixnT fp16.
  - S^T tiles [128 kv, 512 q] fp32 PSUM: 1 fp16 matmul each (kpt slice
    stationary, qpt slice moving). The 1/sqrt(64) scale folds into exp.
  - exp split across two engines: ACT tiles get exact exp -> fp16 est;
    DVE tiles get a Schraudolph fp16 bit-trick (tensor_scalar -> int16,
    bitcast to fp16; ~3% sawtooth err on ~45% of tiles -> ~1.4e-2 end err).
  - PV with est as the STATIONARY operand and a 65-wide moving vp slice
    (64 v-cols + ones): out[q, 0:65] lands in a 128-f32 slot of the mix
    accumulator [128, 2(h), 8(qb), 128] (4 banks; col 64 = sumexp). Each
    tile's 4 PV matmuls hit exactly one bank, so PSUM accumulation groups
    are started/stopped once per bank per lqc (start_tensor_calc clears
    has_written bank-wide).
  - normalization is per-partition (q on partitions): one reciprocal (mix
    col 64) + one tensor_tensor per 1024 q. PE transpose (fp32, identity)
    flips mixn[q,c] -> mixnT[c,q] into a ring psum slot, DVE copies to fp16
    SBUF, one fp16 matmul per 128-q block does the out-projection, DVE
    copies to SBUF, gpsimd DMAs out.
  - PSUM: unified 4-tag ring [128,512] (S tiles, projection psums, transpose
    slots, out-proj psums) + mix (4 banks) = 8 banks.
  - PV emission lags S/exp by 3 tiles so the in-order PE queue never waits
    on an exp in flight; projections for chunk g+1 are interleaved into the
    attention sweep of chunk g (lqc 0); out-projection of lqc L interleaves
    into the first attention steps of lqc L+1.
"""

import numpy as np

import concourse.bacc as bacc
import concourse.mybir as mybir
import concourse.tile as tile
from concourse import bass_utils

P = 128
L = 4096
D = 512
F32 = mybir.dt.float32
F16 = mybir.dt.float16
I16 = mybir.dt.int16
AF = mybir.ActivationFunctionType
MULT = mybir.AluOpType.mult
ADD = mybir.AluOpType.add

# Schraudolph fp16 bit-trick: i16 = A_S * s + B_S, bitcast int16->fp16
# approximates exp(0.125*s). C tuned for min max-rel-err (~3.0%).
A_S = 0.125 * 1477.3195458126642
B_S = 15360.0 - 44.708

_NC = {}


def _dve_tile(lqc, j, h, qh):
    """Which exp tiles take the DVE bit-trick path (rest go to ACT).

    lqc 0 runs lighter on DVE (it also does the projection converts there);
    later lqc's split nearly evenly.
    """
    t = ((lqc * 32 + j) * 2 + h) * 2 + qh
    if lqc == 0:
        return t % 3 == 1  # ~33%
    return (t * 29) % 60 < 29  # ~48%


def build(with_bias=False):
    nc = bacc.Bacc("TRN2", target_bir_lowering=False, debug=False)

    xqt = nc.dram_tensor("xqt", (D, L), F16, kind="ExternalInput").ap()
    xkt = nc.dram_tensor("xkt", (D, L), F16, kind="ExternalInput").ap()
    xvt = nc.dram_tensor("xvt", (D, L), F16, kind="ExternalInput").ap()
    wq = nc.dram_tensor("wq", (D, P), F16, kind="ExternalInput").ap()
    wk = nc.dram_tensor("wk", (D, P), F16, kind="ExternalInput").ap()
    wv = nc.dram_tensor("wv", (D, P), F16, kind="ExternalInput").ap()
    wo = nc.dram_tensor("wo", (P, D), F16, kind="ExternalInput").ap()
    ident = nc.dram_tensor("ident", (P, P), F32, kind="ExternalInput").ap()
    if with_bias:
        bqr = nc.dram_tensor("bqr", (1, P), F16, kind="ExternalInput").ap()
        bkr = nc.dram_tensor("bkr", (1, P), F16, kind="ExternalInput").ap()
        bvr = nc.dram_tensor("bvr", (1, P), F16, kind="ExternalInput").ap()
    out = nc.dram_tensor("out", (L, D), F32, kind="ExternalOutput").ap()

    xqv = xqt.rearrange("(o p) l -> p o l", p=P)
    xkv = xkt.rearrange("(o p) l -> p o l", p=P)
    xvv = xvt.rearrange("(o p) l -> p o l", p=P)

    with tile.TileContext(nc) as tc:
        with tc.tile_pool(name="const", bufs=1) as const, \
             tc.tile_pool(name="persist", bufs=1) as persist, \
             tc.tile_pool(name="pps", bufs=1, space="PSUM") as pps, \
             tc.tile_pool(name="xs", bufs=2) as xs, \
             tc.tile_pool(name="ring", bufs=1, space="PSUM") as ring, \
             tc.tile_pool(name="estp", bufs=6) as estp, \
             tc.tile_pool(name="mixnp", bufs=2) as mixnp, \
             tc.tile_pool(name="smallp", bufs=8) as smallp:

            # ---- constants ----
            wq_sb = const.tile([P, 4, P], F16, tag="wq")
            nc.sync.dma_start(wq_sb, wq.rearrange("(o p) m -> p o m", p=P))
            wk_sb = const.tile([P, 4, P], F16, tag="wk")
            nc.sync.dma_start(wk_sb, wk.rearrange("(o p) m -> p o m", p=P))
            wv_sb = const.tile([P, 4, P], F16, tag="wv")
            nc.sync.dma_start(wv_sb, wv.rearrange("(o p) m -> p o m", p=P))
            wo_sb = const.tile([P, D], F16, tag="wo")
            nc.sync.dma_start(wo_sb, wo)
            id_sb = const.tile([P, P], F32, tag="id")
            nc.sync.dma_start(id_sb, ident)
            if with_bias:
                bq_sb = const.tile([1, P], F16, tag="bq")
                nc.sync.dma_start(bq_sb, bqr)
                bk_sb = const.tile([1, P], F16, tag="bk")
                nc.sync.dma_start(bk_sb, bkr)
                bv_sb = const.tile([1, P], F16, tag="bv")
                nc.sync.dma_start(bv_sb, bvr)
                ones_q = const.tile([1, D], F16, tag="onesq")
                nc.scalar.activation(
                    ones_q, bq_sb[:, 0:1].to_broadcast((1, D)),
                    AF.Identity, bias=1.0, scale=0.0)
                ones_kv = const.tile([1, P], F16, tag="oneskv")
                nc.scalar.activation(
                    ones_kv, bq_sb[:, 0:1].to_broadcast((1, P)),
                    AF.Identity, bias=1.0, scale=0.0)

            # ---- persistent tiles ----
            qpt = persist.tile([P, L], F16, tag="qpt")
            kpt = persist.tile([P, L], F16, tag="kpt")
            vp = persist.tile([P, 32, 130], F16, tag="vp")
            # ones columns (64 for h0 slice 0:65, 129 for h1 slice 65:130)
            nc.scalar.activation(
                vp[:, :, 64:130:65],
                wq_sb[:, 0:1, 0:1].to_broadcast((P, 32, 2)),
                AF.Identity, bias=1.0, scale=0.0)

            # mix[:, h, qb, 0:64] = sum_kv p*v ; mix[:, h, qb, 64] = sumexp
            mix = pps.tile([P, 2, 8, P], F32, tag="mix")      # 4 banks

            ring_n = [0]

            def ring_tile(shape, name):
                t = ring.tile(shape, F32, tag=f"r{ring_n[0] % 4}", name=name)
                ring_n[0] += 1
                return t

            # ---- projection emitters ----
            def proj_x(ch, q_engine, xv_, w_sb, dst, b_sb, xtag, pname):
                sl = slice(ch * D, (ch + 1) * D)
                xt = xs.tile([P, 4, D], F16, tag=xtag)
                q_engine(xt, xv_[:, :, sl])
                ps = ring_tile([P, D], pname)
                for dk in range(4):
                    nc.tensor.matmul(ps, lhsT=w_sb[:, dk, :], rhs=xt[:, dk, :],
                                     start=(dk == 0),
                                     stop=(dk == 3 and not with_bias))
                if with_bias:
                    nc.tensor.matmul(ps, lhsT=b_sb, rhs=ones_q,
                                     start=False, stop=True)
                # convert on DVE: ACT is the scarcer engine during lqc 0
                nc.vector.tensor_copy(dst[:, sl], ps)

            def proj_q(ch, q_engine):
                proj_x(ch, q_engine, xqv, wq_sb, qpt,
                       bq_sb if with_bias else None, "xq", f"pq{ch}")

            def proj_k(ch, q_engine):
                proj_x(ch, q_engine, xkv, wk_sb, kpt,
                       bk_sb if with_bias else None, "xk", f"pk{ch}")

            def proj_v(ch, q_engine):
                sl = slice(ch * D, (ch + 1) * D)
                xt = xs.tile([P, 4, D], F16, tag="xv")
                q_engine(xt, xvv[:, :, sl])
                # one ring tile; 4 sequential accumulation groups in the bank
                ps = ring_tile([P, 4, P], f"pv{ch}")
                for jb in range(4):
                    j = ch * 4 + jb
                    for dk in range(4):
                        nc.tensor.matmul(
                            ps[:, jb, :], lhsT=xt[:, dk, jb * P:(jb + 1) * P],
                            rhs=wv_sb[:, dk, :],
                            start=(dk == 0), stop=(dk == 3 and not with_bias))
                    if with_bias:
                        nc.tensor.matmul(ps[:, jb, :], lhsT=ones_kv, rhs=bv_sb,
                                         start=False, stop=True)
                    # write v channels: h0 -> cols 0:64, h1 -> cols 65:129
                    nc.vector.tensor_copy(_vp_dst(j), ps[:, jb, :])

            def _vp_dst(j):
                # [128, 2, 64] strided view: cols 0:64 and 65:129
                return vp[:, j, 0:130].rearrange("p (h c) -> p h c", c=65)[
                    :, :, 0:64]

            # ---- attention tile helpers ----
            pend = []   # emitted S/exp tiles whose PV is pending

            def emit_s_exp(lqc, j, h, qh):
                st = ring_tile([P, D], f"st{lqc}_{j}_{h}_{qh}")
                q0 = lqc * 1024 + qh * D
                nc.tensor.matmul(
                    st,
                    lhsT=kpt[64 * h:64 * h + 64, j * P:(j + 1) * P],
                    rhs=qpt[64 * h:64 * h + 64, q0:q0 + D],
                    start=True, stop=True)
                if _dve_tile(lqc, j, h, qh):
                    est = estp.tile([P, D], I16, tag="estD",
                                    name=f"eD{lqc}_{j}_{h}_{qh}")
                    nc.vector.tensor_scalar(est, st, A_S, B_S, MULT, ADD)
                    est_f = est.bitcast(F16)
                else:
                    est = estp.tile([P, D], F16, tag="estA",
                                    name=f"eA{lqc}_{j}_{h}_{qh}")
                    nc.scalar.activation(est, st, AF.Exp, scale=0.125)
                    est_f = est
                pend.append((j, h, qh, est_f))

            def emit_pv(j, h, qh, est_f):
                for i in range(4):
                    qb = qh * 4 + i
                    stat = est_f[:, i * P:(i + 1) * P]
                    nc.tensor.matmul(
                        mix[:, h, qb, 0:65], lhsT=stat,
                        rhs=vp[:, j, 65 * h:65 * h + 65],
                        start=(j == 0 and i == 0), stop=(j == 31 and i == 3))

            def flush_pv(keep):
                while len(pend) > keep:
                    emit_pv(*pend.pop(0))

            # ---- out-projection (for finished lqc) ----
            def emit_outproj_head(lqc):
                # per-head recip+TT so each mix bank pair is released as
                # soon as its own normalize has read it
                mixn = mixnp.tile([P, 8, 2, 64], F32, tag="mixn",
                                  name=f"mixn{lqc}")
                for h in range(2):
                    rb = smallp.tile([P, 8], F32, tag=f"rb{h}",
                                     name=f"rb{lqc}_{h}")
                    nc.vector.reciprocal(rb, mix[:, h, :, 64])
                    nc.vector.tensor_tensor(
                        mixn[:, :, h, :].rearrange("p q c -> p q c"),
                        mix[:, h, :, 0:64],
                        rb[:, :, None].to_broadcast((P, 8, 64)), MULT)
                return mixn

            def outproj_stage1(lqc, qb, mixn):
                tr = ring_tile([P, P], f"tr{lqc}_{qb}")
                nc.tensor.transpose(
                    tr, mixn[:, qb].rearrange("p h c -> p (h c)"), id_sb)
                mt = smallp.tile([P, P], F16, tag="mt", name=f"mt{lqc}_{qb}")
                if qb % 2 == 0:
                    nc.vector.tensor_copy(mt, tr)
                else:
                    nc.scalar.activation(mt, tr, AF.Identity)
                return mt

            def outproj_stage2(lqc, qb, mt):
                po = ring_tile([P, D], f"po{lqc}_{qb}")
                nc.tensor.matmul(po, lhsT=mt, rhs=wo_sb, start=True, stop=True)
                ob = smallp.tile([P, D], F32, tag="ob", name=f"ob{lqc}_{qb}")
                if qb % 2 == 0:
                    nc.scalar.activation(ob, po, AF.Identity)
                else:
                    nc.vector.tensor_copy(ob, po)
                r0 = lqc * 1024 + qb * P
                if qb % 2 == 0:
                    nc.gpsimd.dma_start(out[r0:r0 + P, :], ob)
                else:
                    nc.sync.dma_start(out[r0:r0 + P, :], ob)

            # =================== emission ===================
            proj_q(0, nc.sync.dma_start)
            proj_q(1, nc.gpsimd.dma_start)
            proj_k(0, nc.sync.dma_start)
            proj_v(0, nc.gpsimd.dma_start)

            prev_mixn = None
            prev_mt = {}
            for lqc in range(4):
                for j in range(32):
                    if lqc == 0:
                        g, r = j // 4, j % 4
                        if r == 0 and g + 1 < 8:
                            proj_k(g + 1, nc.sync.dma_start)
                            proj_v(g + 1, nc.gpsimd.dma_start)
                        if r == 2 and 1 <= g <= 3:
                            proj_q(2 * g, nc.sync.dma_start)
                            proj_q(2 * g + 1, nc.gpsimd.dma_start)
                    elif prev_mixn is not None and 2 <= j <= 25:
                        # spread previous lqc's out-projection: each qb's
                        # stage1 at j=2+3qb, stage2 one j later (deps stale
                        # by a full attention step -> no engine blocking)
                        if (j - 2) % 3 == 0 and (j - 2) // 3 < 8:
                            qb = (j - 2) // 3
                            prev_mt[qb] = outproj_stage1(lqc - 1, qb, prev_mixn)
                        elif (j - 3) % 3 == 0 and (j - 3) // 3 < 8:
                            qb = (j - 3) // 3
                            outproj_stage2(lqc - 1, qb, prev_mt.pop(qb))
                    for h in range(2):
                        for qh in range(2):
                            emit_s_exp(lqc, j, h, qh)
                            flush_pv(keep=3)
                flush_pv(keep=0)
                prev_mixn = emit_outproj_head(lqc)
            # tail: wave-ordered so each engine runs its 8 ops back-to-back
            mts = [outproj_stage1(3, qb, prev_mixn) for qb in range(8)]
            for qb in range(8):
                outproj_stage2(3, qb, mts[qb])

    nc.compile()
    return nc


def get_nc(with_bias=False):
    if with_bias not in _NC:
        _NC[with_bias] = build(with_bias)
    return _NC[with_bias]


def make_in_maps(q, k, v, Wq, bq, Wk, bk, Wv, bv, Wo, bo, with_bias=False):
    q = np.asarray(q, np.float32)
    k = np.asarray(k, np.float32)
    v = np.asarray(v, np.float32)
    Wq = np.asarray(Wq, np.float16)
    Wk = np.asarray(Wk, np.float16)
    Wv = np.asarray(Wv, np.float16)
    Wo = np.asarray(Wo, np.float16)
    ident = np.eye(P, dtype=np.float32)
    xts = {}
    for n in range(2):
        xts[n] = (np.ascontiguousarray(q[n].T).astype(np.float16),
                  np.ascontiguousarray(k[n].T).astype(np.float16),
                  np.ascontiguousarray(v[n].T).astype(np.float16))
    in_maps = []
    for c in range(8):
        n, hp = c // 4, c % 4
        sl = slice(P * hp, P * (hp + 1))
        xq, xk, xv = xts[n]
        m = {
            "xqt": xq, "xkt": xk, "xvt": xv,
            "wq": np.ascontiguousarray(Wq[:, sl]),
            "wk": np.ascontiguousarray(Wk[:, sl]),
            "wv": np.ascontiguousarray(Wv[:, sl]),
            "wo": np.ascontiguousarray(Wo[sl, :]),
            "ident": ident,
        }
        if with_bias:
            m["bqr"] = np.asarray(bq, np.float16)[sl].reshape(1, P)
            m["bkr"] = np.asarray(bk, np.float16)[sl].reshape(1, P)
            m["bvr"] = np.asarray(bv, np.float16)[sl].reshape(1, P)
        in_maps.append(m)
    return in_maps


def assemble(results, bo):
    bo = np.asarray(bo, np.float32)
    out = np.zeros((2, L, D), np.float32)
    for c in range(8):
        out[c // 4] += results[c]["out"]
    out += bo[None, None, :]
    return out


def kernel(q, k, v, Wq, bq, Wk, bk, Wv, bv, Wo, bo):
    with_bias = any(
        np.any(np.asarray(b)) for b in (bq, bk, bv))
    nc = get_nc(with_bias)
    in_maps = make_in_maps(q, k, v, Wq, bq, Wk, bk, Wv, bv, Wo, bo,
                           with_bias=with_bias)
    res = bass_utils.run_bass_kernel_spmd(nc, in_maps, core_ids=list(range(8)))
    return assemble(res.results, bo)


if __name__ == "__main__":
    build()
    print("build ok")


# revision 30
# speedup vs baseline: 1.6659x; 1.0161x over previous
"""Multi-head attention (q/k/v projections + softmax attention + out-projection)
on 8 Trainium2 NeuronCores.

Sharding: 16 (batch, head) units over 8 cores -> core c handles batch n = c//4
and head pair hp = c%4 (channels 128*hp : 128*hp+128). Host sums the 4 partial
outputs per batch and adds bo.

v4 design (baseline was 418us):
  - fp16 operand universe: x, W, qpt/kpt [128ch, 4096] fp16, vp [128kv, 32,
    130] fp16 (cols 64/129 = ones for sumexp), Wo fp16, mixnT fp16.
  - S^T tiles [128 kv, 512 q] fp32 PSUM: 1 fp16 matmul each (kpt slice
    stationary, qpt slice moving). The 1/sqrt(64) scale folds into exp.
  - exp split across two engines: ACT tiles get exact exp -> fp16 est;
    DVE tiles get a Schraudolph fp16 bit-trick (tensor_scalar -> int16,
    bitcast to fp16; ~3% sawtooth err on ~45% of tiles -> ~1.4e-2 end err).
  - PV with est as the STATIONARY operand and a 65-wide moving vp slice
    (64 v-cols + ones): out[q, 0:65] lands in a 128-f32 slot of the mix
    accumulator [128, 2(h), 8(qb), 128] (4 banks; col 64 = sumexp). Each
    tile's 4 PV matmuls hit exactly one bank, so PSUM accumulation groups
    are started/stopped once per bank per lqc (start_tensor_calc clears
    has_written bank-wide).
  - normalization is per-partition (q on partitions): one reciprocal (mix
    col 64) + one tensor_tensor per 1024 q. PE transpose (fp32, identity)
    flips mixn[q,c] -> mixnT[c,q] into a ring psum slot, DVE copies to fp16
    SBUF, one fp16 matmul per 128-q block does the out-projection, DVE
    copies to SBUF, gpsimd DMAs out.
  - PSUM: unified 4-tag ring [128,512] (S tiles, projection psums, transpose
    slots, out-proj psums) + mix (4 banks) = 8 banks.
  - PV emission lags S/exp by 3 tiles so the in-order PE queue never waits
    on an exp in flight; projections for chunk g+1 are interleaved into the
    attention sweep of chunk g (lqc 0); out-projection of lqc L interleaves
    into the first attention steps of lqc L+1.
"""

import numpy as np

import concourse.bacc as bacc
import concourse.mybir as mybir
import concourse.tile as tile
from concourse import bass_utils

P = 128
L = 4096
D = 512
F32 = mybir.dt.float32
F16 = mybir.dt.float16
I16 = mybir.dt.int16
AF = mybir.ActivationFunctionType
MULT = mybir.AluOpType.mult
ADD = mybir.AluOpType.add

# Schraudolph fp16 bit-trick: i16 = A_S * s + B_S, bitcast int16->fp16
# approximates exp(0.125*s). C tuned for min max-rel-err (~3.0%).
A_S = 0.125 * 1477.3195458126642
B_S = 15360.0 - 56.0

_NC = {}


def _dve_tile(lqc, j, h, qh):
    """Which exp tiles take the DVE bit-trick path (rest go to ACT).

    lqc 0 is PE-bound (projections) and ACT has spare capacity there, so it
    runs a 1/4 DVE split; steady lqc's run 5/12 to balance ACT/DVE/PE.
    """
    t = ((lqc * 32 + j) * 2 + h) * 2 + qh
    if lqc == 0:
        return t % 4 == 1
    return (t * 9) % 20 < 9


def build(with_bias=False):
    nc = bacc.Bacc("TRN2", target_bir_lowering=False, debug=False)

    xqt = nc.dram_tensor("xqt", (D, L), F16, kind="ExternalInput").ap()
    xkt = nc.dram_tensor("xkt", (D, L), F16, kind="ExternalInput").ap()
    xvt = nc.dram_tensor("xvt", (D, L), F16, kind="ExternalInput").ap()
    wq = nc.dram_tensor("wq", (D, P), F16, kind="ExternalInput").ap()
    wk = nc.dram_tensor("wk", (D, P), F16, kind="ExternalInput").ap()
    wv = nc.dram_tensor("wv", (D, P), F16, kind="ExternalInput").ap()
    wo = nc.dram_tensor("wo", (P, D), F16, kind="ExternalInput").ap()
    ident = nc.dram_tensor("ident", (P, P), F32, kind="ExternalInput").ap()
    if with_bias:
        bqr = nc.dram_tensor("bqr", (1, P), F16, kind="ExternalInput").ap()
        bkr = nc.dram_tensor("bkr", (1, P), F16, kind="ExternalInput").ap()
        bvr = nc.dram_tensor("bvr", (1, P), F16, kind="ExternalInput").ap()
    out = nc.dram_tensor("out", (L, D), F32, kind="ExternalOutput").ap()

    xqv = xqt.rearrange("(o p) l -> p o l", p=P)
    xkv = xkt.rearrange("(o p) l -> p o l", p=P)
    xvv = xvt.rearrange("(o p) l -> p o l", p=P)

    with tile.TileContext(nc) as tc:
        with tc.tile_pool(name="const", bufs=1) as const, \
             tc.tile_pool(name="persist", bufs=1) as persist, \
             tc.tile_pool(name="pps", bufs=1, space="PSUM") as pps, \
             tc.tile_pool(name="xs", bufs=2) as xs, \
             tc.tile_pool(name="ring", bufs=1, space="PSUM") as ring, \
             tc.tile_pool(name="estp", bufs=16) as estp, \
             tc.tile_pool(name="mixnp", bufs=2) as mixnp, \
             tc.tile_pool(name="smallp", bufs=8) as smallp:

            # ---- constants ----
            wq_sb = const.tile([P, 4, P], F16, tag="wq")
            nc.sync.dma_start(wq_sb, wq.rearrange("(o p) m -> p o m", p=P))
            wk_sb = const.tile([P, 4, P], F16, tag="wk")
            nc.sync.dma_start(wk_sb, wk.rearrange("(o p) m -> p o m", p=P))
            wv_sb = const.tile([P, 4, P], F16, tag="wv")
            nc.sync.dma_start(wv_sb, wv.rearrange("(o p) m -> p o m", p=P))
            wo_sb = const.tile([P, D], F16, tag="wo")
            nc.sync.dma_start(wo_sb, wo)
            id_sb = const.tile([P, P], F32, tag="id")
            nc.sync.dma_start(id_sb, ident)
            if with_bias:
                bq_sb = const.tile([1, P], F16, tag="bq")
                nc.sync.dma_start(bq_sb, bqr)
                bk_sb = const.tile([1, P], F16, tag="bk")
                nc.sync.dma_start(bk_sb, bkr)
                bv_sb = const.tile([1, P], F16, tag="bv")
                nc.sync.dma_start(bv_sb, bvr)
                ones_q = const.tile([1, D], F16, tag="onesq")
                nc.scalar.activation(
                    ones_q, bq_sb[:, 0:1].to_broadcast((1, D)),
                    AF.Identity, bias=1.0, scale=0.0)
                ones_kv = const.tile([1, P], F16, tag="oneskv")
                nc.scalar.activation(
                    ones_kv, bq_sb[:, 0:1].to_broadcast((1, P)),
                    AF.Identity, bias=1.0, scale=0.0)

            # ---- persistent tiles ----
            qpt = persist.tile([P, L], F16, tag="qpt")
            kpt = persist.tile([P, L], F16, tag="kpt")
            vp = persist.tile([P, 32, 130], F16, tag="vp")
            # ones columns (64 for h0 slice 0:65, 129 for h1 slice 65:130)
            nc.scalar.activation(
                vp[:, :, 64:130:65],
                wq_sb[:, 0:1, 0:1].to_broadcast((P, 32, 2)),
                AF.Identity, bias=1.0, scale=0.0)

            # mix[:, h, qb, 0:64] = sum_kv p*v ; mix[:, h, qb, 64] = sumexp
            mix = pps.tile([P, 2, 8, P], F32, tag="mix")      # 4 banks

            ring_n = [0]

            def ring_tile(shape, name):
                t = ring.tile(shape, F32, tag=f"r{ring_n[0] % 4}", name=name)
                ring_n[0] += 1
                return t

            # ---- projection emitters ----
            def proj_x(ch, q_engine, xv_, w_sb, dst, b_sb, xtag, pname):
                sl = slice(ch * D, (ch + 1) * D)
                xt = xs.tile([P, 4, D], F16, tag=xtag)
                q_engine(xt, xv_[:, :, sl])
                ps = ring_tile([P, D], pname)
                for dk in range(4):
                    nc.tensor.matmul(ps, lhsT=w_sb[:, dk, :], rhs=xt[:, dk, :],
                                     start=(dk == 0),
                                     stop=(dk == 3 and not with_bias))
                if with_bias:
                    nc.tensor.matmul(ps, lhsT=b_sb, rhs=ones_q,
                                     start=False, stop=True)
                # convert on DVE: ACT is the scarcer engine during lqc 0
                nc.vector.tensor_copy(dst[:, sl], ps)

            def proj_q(ch, q_engine):
                proj_x(ch, q_engine, xqv, wq_sb, qpt,
                       bq_sb if with_bias else None, "xq", f"pq{ch}")

            def proj_k(ch, q_engine):
                proj_x(ch, q_engine, xkv, wk_sb, kpt,
                       bk_sb if with_bias else None, "xk", f"pk{ch}")

            def proj_v(ch, q_engine):
                sl = slice(ch * D, (ch + 1) * D)
                xt = xs.tile([P, 4, D], F16, tag="xv")
                q_engine(xt, xvv[:, :, sl])
                # one ring tile; 4 sequential accumulation groups in the bank
                ps = ring_tile([P, 4, P], f"pv{ch}")
                for jb in range(4):
                    j = ch * 4 + jb
                    for dk in range(4):
                        nc.tensor.matmul(
                            ps[:, jb, :], lhsT=xt[:, dk, jb * P:(jb + 1) * P],
                            rhs=wv_sb[:, dk, :],
                            start=(dk == 0), stop=(dk == 3 and not with_bias))
                    if with_bias:
                        nc.tensor.matmul(ps[:, jb, :], lhsT=ones_kv, rhs=bv_sb,
                                         start=False, stop=True)
                    # write v channels: h0 -> cols 0:64, h1 -> cols 65:129
                    nc.vector.tensor_copy(_vp_dst(j), ps[:, jb, :])

            def _vp_dst(j):
                # [128, 2, 64] strided view: cols 0:64 and 65:129
                return vp[:, j, 0:130].rearrange("p (h c) -> p h c", c=65)[
                    :, :, 0:64]

            # ---- attention tile helpers ----
            pend = []   # emitted S/exp tiles whose PV is pending

            def emit_s_exp(lqc, j, h, qh):
                st = ring_tile([P, D], f"st{lqc}_{j}_{h}_{qh}")
                q0 = lqc * 1024 + qh * D
                nc.tensor.matmul(
                    st,
                    lhsT=kpt[64 * h:64 * h + 64, j * P:(j + 1) * P],
                    rhs=qpt[64 * h:64 * h + 64, q0:q0 + D],
                    start=True, stop=True)
                if _dve_tile(lqc, j, h, qh):
                    est = estp.tile([P, D], I16, tag="estD",
                                    name=f"eD{lqc}_{j}_{h}_{qh}")
                    nc.vector.tensor_scalar(est, st, A_S, B_S, MULT, ADD)
                    est_f = est.bitcast(F16)
                else:
                    est = estp.tile([P, D], F16, tag="estA",
                                    name=f"eA{lqc}_{j}_{h}_{qh}")
                    nc.scalar.activation(est, st, AF.Exp, scale=0.125)
                    est_f = est
                pend.append((j, h, qh, est_f))

            def emit_pv(j, h, qh, est_f):
                for i in range(4):
                    qb = qh * 4 + i
                    stat = est_f[:, i * P:(i + 1) * P]
                    nc.tensor.matmul(
                        mix[:, h, qb, 0:65], lhsT=stat,
                        rhs=vp[:, j, 65 * h:65 * h + 65],
                        start=(j == 0 and i == 0), stop=(j == 31 and i == 3))

            def flush_pv(keep):
                while len(pend) > keep:
                    emit_pv(*pend.pop(0))

            # ---- out-projection (for finished lqc) ----
            def emit_outproj_head(lqc):
                # per-head recip+TT so each mix bank pair is released as
                # soon as its own normalize has read it; fp16 out so the
                # transpose can go through the DMA xbar (2-byte only)
                mixn = mixnp.tile([P, 8, 2, 64], F16, tag="mixn",
                                  name=f"mixn{lqc}")
                for h in range(2):
                    rb = smallp.tile([P, 8], F32, tag=f"rb{h}",
                                     name=f"rb{lqc}_{h}")
                    nc.vector.reciprocal(rb, mix[:, h, :, 64])
                    nc.vector.tensor_tensor(
                        mixn[:, :, h, :], mix[:, h, :, 0:64],
                        rb[:, :, None].to_broadcast((P, 8, 64)), MULT)
                return mixn

            def outproj_stage1(lqc, qb, mixn, tail=False):
                # transpose through the DMA xbar: no psum slot, no PE, no copy
                mt = smallp.tile([P, P], F16, tag="mt", name=f"mt{lqc}_{qb}")
                eng = (nc.sync, nc.scalar)[qb % 2] if tail else nc.sync
                eng.dma_start_transpose(
                    mt, mixn[:, qb].rearrange("p h c -> p (h c)"))
                return mt

            def outproj_stage2(lqc, qb, mt):
                po = ring_tile([P, D], f"po{lqc}_{qb}")
                nc.tensor.matmul(po, lhsT=mt, rhs=wo_sb, start=True, stop=True)
                ob = smallp.tile([P, D], F32, tag="ob", name=f"ob{lqc}_{qb}")
                nc.vector.tensor_copy(ob, po)
                r0 = lqc * 1024 + qb * P
                if qb % 2 == 0:
                    nc.gpsimd.dma_start(out[r0:r0 + P, :], ob)
                else:
                    nc.sync.dma_start(out[r0:r0 + P, :], ob)

            # =================== emission ===================
            proj_q(0, nc.sync.dma_start)
            proj_q(1, nc.gpsimd.dma_start)
            proj_k(0, nc.sync.dma_start)
            proj_v(0, nc.gpsimd.dma_start)

            prev_mixn = None
            prev_mt = {}
            for lqc in range(4):
                for j in range(32):
                    if lqc == 0:
                        g, r = j // 4, j % 4
                        if r == 0 and g + 1 < 8:
                            proj_k(g + 1, nc.sync.dma_start)
                            proj_v(g + 1, nc.gpsimd.dma_start)
                        if r == 2 and 1 <= g <= 3:
                            proj_q(2 * g, nc.sync.dma_start)
                            proj_q(2 * g + 1, nc.gpsimd.dma_start)
                    elif prev_mixn is not None:
                        # spread previous lqc's out-projection: transposes
                        # (DMA xbar, no ring) at j=1..8; out-proj matmuls in
                        # parity-preserving ring pairs at j=12,15,18,21
                        if 1 <= j <= 8:
                            prev_mt[j - 1] = outproj_stage1(
                                lqc - 1, j - 1, prev_mixn)
                        elif j in (12, 15, 18, 21):
                            i = (j - 12) // 3
                            outproj_stage2(lqc - 1, 2 * i, prev_mt.pop(2 * i))
                            outproj_stage2(lqc - 1, 2 * i + 1,
                                           prev_mt.pop(2 * i + 1))
                    for h in range(2):
                        for qh in range(2):
                            emit_s_exp(lqc, j, h, qh)
                            flush_pv(keep=12)
                flush_pv(keep=0)
                prev_mixn = emit_outproj_head(lqc)
            # tail: wave-ordered so each engine runs its 8 ops back-to-back
            mts = [outproj_stage1(3, qb, prev_mixn, tail=True)
                   for qb in range(8)]
            for qb in range(8):
                outproj_stage2(3, qb, mts[qb])


    nc.compile()
    return nc


def get_nc(with_bias=False):
    if with_bias not in _NC:
        _NC[with_bias] = build(with_bias)
    return _NC[with_bias]


def make_in_maps(q, k, v, Wq, bq, Wk, bk, Wv, bv, Wo, bo, with_bias=False):
    q = np.asarray(q, np.float32)
    k = np.asarray(k, np.float32)
    v = np.asarray(v, np.float32)
    Wq = np.asarray(Wq, np.float16)
    Wk = np.asarray(Wk, np.float16)
    Wv = np.asarray(Wv, np.float16)
    Wo = np.asarray(Wo, np.float16)
    ident = np.eye(P, dtype=np.float32)
    xts = {}
    for n in range(2):
        xts[n] = (np.ascontiguousarray(q[n].T).astype(np.float16),
                  np.ascontiguousarray(k[n].T).astype(np.float16),
                  np.ascontiguousarray(v[n].T).astype(np.float16))
    in_maps = []
    for c in range(8):
        n, hp = c // 4, c % 4
        sl = slice(P * hp, P * (hp + 1))
        xq, xk, xv = xts[n]
        m = {
            "xqt": xq, "xkt": xk, "xvt": xv,
            "wq": np.ascontiguousarray(Wq[:, sl]),
            "wk": np.ascontiguousarray(Wk[:, sl]),
            "wv": np.ascontiguousarray(Wv[:, sl]),
            "wo": np.ascontiguousarray(Wo[sl, :]),
            "ident": ident,
        }
        if with_bias:
            m["bqr"] = np.asarray(bq, np.float16)[sl].reshape(1, P)
            m["bkr"] = np.asarray(bk, np.float16)[sl].reshape(1, P)
            m["bvr"] = np.asarray(bv, np.float16)[sl].reshape(1, P)
        in_maps.append(m)
    return in_maps


def assemble(results, bo):
    bo = np.asarray(bo, np.float32)
    out = np.zeros((2, L, D), np.float32)
    for c in range(8):
        out[c // 4] += results[c]["out"]
    out += bo[None, None, :]
    return out


def kernel(q, k, v, Wq, bq, Wk, bk, Wv, bv, Wo, bo):
    with_bias = any(
        np.any(np.asarray(b)) for b in (bq, bk, bv))
    nc = get_nc(with_bias)
    in_maps = make_in_maps(q, k, v, Wq, bq, Wk, bk, Wv, bv, Wo, bo,
                           with_bias=with_bias)
    res = bass_utils.run_bass_kernel_spmd(nc, in_maps, core_ids=list(range(8)))
    return assemble(res.results, bo)


if __name__ == "__main__":
    build()
    print("build ok")


# revision 34
# speedup vs baseline: 1.6848x; 1.0113x over previous
"""Multi-head attention (q/k/v projections + softmax attention + out-projection)
on 8 Trainium2 NeuronCores.

Sharding: 16 (batch, head) units over 8 cores -> core c handles batch n = c//4
and head pair hp = c%4 (channels 128*hp : 128*hp+128). Host sums the 4 partial
outputs per batch and adds bo.

v4 design (baseline was 418us):
  - fp16 operand universe: x, W, qpt/kpt [128ch, 4096] fp16, vp [128kv, 32,
    130] fp16 (cols 64/129 = ones for sumexp), Wo fp16, mixnT fp16.
  - S^T tiles [128 kv, 512 q] fp32 PSUM: 1 fp16 matmul each (kpt slice
    stationary, qpt slice moving). The 1/sqrt(64) scale folds into exp.
  - exp split across two engines: ACT tiles get exact exp -> fp16 est;
    DVE tiles get a Schraudolph fp16 bit-trick (tensor_scalar -> int16,
    bitcast to fp16; ~3% sawtooth err on ~45% of tiles -> ~1.4e-2 end err).
  - PV with est as the STATIONARY operand and a 65-wide moving vp slice
    (64 v-cols + ones): out[q, 0:65] lands in a 128-f32 slot of the mix
    accumulator [128, 2(h), 8(qb), 128] (4 banks; col 64 = sumexp). Each
    tile's 4 PV matmuls hit exactly one bank, so PSUM accumulation groups
    are started/stopped once per bank per lqc (start_tensor_calc clears
    has_written bank-wide).
  - normalization is per-partition (q on partitions): one reciprocal (mix
    col 64) + one tensor_tensor per 1024 q. PE transpose (fp32, identity)
    flips mixn[q,c] -> mixnT[c,q] into a ring psum slot, DVE copies to fp16
    SBUF, one fp16 matmul per 128-q block does the out-projection, DVE
    copies to SBUF, gpsimd DMAs out.
  - PSUM: unified 4-tag ring [128,512] (S tiles, projection psums, transpose
    slots, out-proj psums) + mix (4 banks) = 8 banks.
  - PV emission lags S/exp by 3 tiles so the in-order PE queue never waits
    on an exp in flight; projections for chunk g+1 are interleaved into the
    attention sweep of chunk g (lqc 0); out-projection of lqc L interleaves
    into the first attention steps of lqc L+1.
"""

import numpy as np

import concourse.bacc as bacc
import concourse.mybir as mybir
import concourse.tile as tile
from concourse import bass_utils

P = 128
L = 4096
D = 512
F32 = mybir.dt.float32
F16 = mybir.dt.float16
I16 = mybir.dt.int16
AF = mybir.ActivationFunctionType
MULT = mybir.AluOpType.mult
ADD = mybir.AluOpType.add

# Schraudolph fp16 bit-trick: i16 = A_S * s + B_S, bitcast int16->fp16
# approximates exp(0.125*s). C tuned for min max-rel-err (~3.0%).
A_S = 0.125 * 1477.3195458126642
B_S = 15360.0 - 56.0

_NC = {}


def _dve_tile(t, block0):
    """Which exp tiles take the DVE bit-trick path (rest go to ACT).

    Block 0 is PE-bound (projections) and ACT has spare capacity there, so
    it runs a 1/4 DVE split; later blocks run ~0.45 to balance ACT/DVE/PE.
    """
    if block0 and t < 64:
        return t % 4 == 1       # projections + DMA dominate: light DVE
    if block0:
        return (t * 5) % 12 < 5  # late block 0: relieve the ACT backlog
    return (t * 9) % 20 < 9


def build(with_bias=False):
    nc = bacc.Bacc("TRN2", target_bir_lowering=False, debug=False)

    xqt = nc.dram_tensor("xqt", (D, L), F16, kind="ExternalInput").ap()
    xkt = nc.dram_tensor("xkt", (D, L), F16, kind="ExternalInput").ap()
    xvt = nc.dram_tensor("xvt", (D, L), F16, kind="ExternalInput").ap()
    wq = nc.dram_tensor("wq", (D, P), F16, kind="ExternalInput").ap()
    wk = nc.dram_tensor("wk", (D, P), F16, kind="ExternalInput").ap()
    wv = nc.dram_tensor("wv", (D, P), F16, kind="ExternalInput").ap()
    wo = nc.dram_tensor("wo", (P, D), F16, kind="ExternalInput").ap()
    ident = nc.dram_tensor("ident", (P, P), F32, kind="ExternalInput").ap()
    if with_bias:
        bqr = nc.dram_tensor("bqr", (1, P), F16, kind="ExternalInput").ap()
        bkr = nc.dram_tensor("bkr", (1, P), F16, kind="ExternalInput").ap()
        bvr = nc.dram_tensor("bvr", (1, P), F16, kind="ExternalInput").ap()
    out = nc.dram_tensor("out", (L, D), F32, kind="ExternalOutput").ap()

    xqv = xqt.rearrange("(o p) l -> p o l", p=P)
    xkv = xkt.rearrange("(o p) l -> p o l", p=P)
    xvv = xvt.rearrange("(o p) l -> p o l", p=P)

    with tile.TileContext(nc) as tc:
        with tc.tile_pool(name="const", bufs=1) as const, \
             tc.tile_pool(name="persist", bufs=1) as persist, \
             tc.tile_pool(name="pps", bufs=1, space="PSUM") as pps, \
             tc.tile_pool(name="xs", bufs=2) as xs, \
             tc.tile_pool(name="ring", bufs=1, space="PSUM") as ring, \
             tc.tile_pool(name="estp", bufs=16) as estp, \
             tc.tile_pool(name="mixnp", bufs=2) as mixnp, \
             tc.tile_pool(name="smallp", bufs=8) as smallp:

            # ---- constants ----
            wq_sb = const.tile([P, 4, P], F16, tag="wq")
            nc.sync.dma_start(wq_sb, wq.rearrange("(o p) m -> p o m", p=P))
            wk_sb = const.tile([P, 4, P], F16, tag="wk")
            nc.sync.dma_start(wk_sb, wk.rearrange("(o p) m -> p o m", p=P))
            wv_sb = const.tile([P, 4, P], F16, tag="wv")
            nc.sync.dma_start(wv_sb, wv.rearrange("(o p) m -> p o m", p=P))
            wo_sb = const.tile([P, D], F16, tag="wo")
            nc.sync.dma_start(wo_sb, wo)
            id_sb = const.tile([P, P], F32, tag="id")
            nc.sync.dma_start(id_sb, ident)
            if with_bias:
                bq_sb = const.tile([1, P], F16, tag="bq")
                nc.sync.dma_start(bq_sb, bqr)
                bk_sb = const.tile([1, P], F16, tag="bk")
                nc.sync.dma_start(bk_sb, bkr)
                bv_sb = const.tile([1, P], F16, tag="bv")
                nc.sync.dma_start(bv_sb, bvr)
                ones_q = const.tile([1, D], F16, tag="onesq")
                nc.scalar.activation(
                    ones_q, bq_sb[:, 0:1].to_broadcast((1, D)),
                    AF.Identity, bias=1.0, scale=0.0)
                ones_kv = const.tile([1, P], F16, tag="oneskv")
                nc.scalar.activation(
                    ones_kv, bq_sb[:, 0:1].to_broadcast((1, P)),
                    AF.Identity, bias=1.0, scale=0.0)

            # ---- persistent tiles ----
            qpt = persist.tile([P, L], F16, tag="qpt")
            kpt = persist.tile([P, L], F16, tag="kpt")
            vp = persist.tile([P, 32, 130], F16, tag="vp")
            # ones columns (64 for h0 slice 0:65, 129 for h1 slice 65:130)
            nc.scalar.activation(
                vp[:, :, 64:130:65],
                wq_sb[:, 0:1, 0:1].to_broadcast((P, 32, 2)),
                AF.Identity, bias=1.0, scale=0.0)

            # mix[:, h, qb, 0:64] = sum_kv p*v ; mix[:, h, qb, 64] = sumexp
            mix = pps.tile([P, 2, 8, P], F32, tag="mix")      # 4 banks

            ring_n = [0]

            def ring_tile(shape, name):
                t = ring.tile(shape, F32, tag=f"r{ring_n[0] % 4}", name=name)
                ring_n[0] += 1
                return t

            # ---- projection emitters ----
            def proj_x(ch, q_engine, xv_, w_sb, dst, b_sb, xtag, pname):
                sl = slice(ch * D, (ch + 1) * D)
                xt = xs.tile([P, 4, D], F16, tag=xtag)
                q_engine(xt, xv_[:, :, sl])
                ps = ring_tile([P, D], pname)
                for dk in range(4):
                    nc.tensor.matmul(ps, lhsT=w_sb[:, dk, :], rhs=xt[:, dk, :],
                                     start=(dk == 0),
                                     stop=(dk == 3 and not with_bias))
                if with_bias:
                    nc.tensor.matmul(ps, lhsT=b_sb, rhs=ones_q,
                                     start=False, stop=True)
                # convert on DVE: ACT is the scarcer engine during lqc 0
                nc.vector.tensor_copy(dst[:, sl], ps)

            def proj_q(ch, q_engine):
                proj_x(ch, q_engine, xqv, wq_sb, qpt,
                       bq_sb if with_bias else None, "xq", f"pq{ch}")

            def proj_k(ch, q_engine):
                proj_x(ch, q_engine, xkv, wk_sb, kpt,
                       bk_sb if with_bias else None, "xk", f"pk{ch}")

            def proj_v(ch, q_engine):
                sl = slice(ch * D, (ch + 1) * D)
                xt = xs.tile([P, 4, D], F16, tag="xv")
                q_engine(xt, xvv[:, :, sl])
                # one ring tile; 4 sequential accumulation groups in the bank
                ps = ring_tile([P, 4, P], f"pv{ch}")
                for jb in range(4):
                    j = ch * 4 + jb
                    for dk in range(4):
                        nc.tensor.matmul(
                            ps[:, jb, :], lhsT=xt[:, dk, jb * P:(jb + 1) * P],
                            rhs=wv_sb[:, dk, :],
                            start=(dk == 0), stop=(dk == 3 and not with_bias))
                    if with_bias:
                        nc.tensor.matmul(ps[:, jb, :], lhsT=ones_kv, rhs=bv_sb,
                                         start=False, stop=True)
                    # write v channels: h0 -> cols 0:64, h1 -> cols 65:129
                    nc.vector.tensor_copy(_vp_dst(j), ps[:, jb, :])

            def _vp_dst(j):
                # [128, 2, 64] strided view: cols 0:64 and 65:129
                return vp[:, j, 0:130].rearrange("p (h c) -> p h c", c=65)[
                    :, :, 0:64]

            # ---- attention tile helpers ----
            pend = []   # emitted S/exp tiles whose PV is pending

            tcount = [0]

            def emit_s_exp(bq0, block0, j, h, qh):
                t = tcount[0]
                tcount[0] += 1
                st = ring_tile([P, D], f"st{t}")
                q0 = bq0 + qh * D
                nc.tensor.matmul(
                    st,
                    lhsT=kpt[64 * h:64 * h + 64, j * P:(j + 1) * P],
                    rhs=qpt[64 * h:64 * h + 64, q0:q0 + D],
                    start=True, stop=True)
                if _dve_tile(t, block0):
                    est = estp.tile([P, D], I16, tag="estD", name=f"eD{t}")
                    nc.vector.tensor_scalar(est, st, A_S, B_S, MULT, ADD)
                    est_f = est.bitcast(F16)
                else:
                    est = estp.tile([P, D], F16, tag="estA", name=f"eA{t}")
                    nc.scalar.activation(est, st, AF.Exp, scale=0.125)
                    est_f = est
                pend.append((j, h, qh, est_f))

            def emit_pv(j, h, qh, est_f):
                for i in range(4):
                    qb = qh * 4 + i
                    stat = est_f[:, i * P:(i + 1) * P]
                    nc.tensor.matmul(
                        mix[:, h, qb, 0:65], lhsT=stat,
                        rhs=vp[:, j, 65 * h:65 * h + 65],
                        start=(j == 0 and i == 0), stop=(j == 31 and i == 3))

            def flush_pv(keep):
                while len(pend) > keep:
                    emit_pv(*pend.pop(0))

            # ---- out-projection (for finished lqc) ----
            def emit_outproj_head(bi, nqb):
                # per-head recip+TT so each mix bank pair is released as
                # soon as its own normalize has read it; fp16 out so the
                # transpose can go through the DMA xbar (2-byte only)
                mixn = mixnp.tile([P, 8, 2, 64], F16, tag="mixn",
                                  name=f"mixn{bi}")
                for h in range(2):
                    rb = smallp.tile([P, 8], F32, tag=f"rb{h}",
                                     name=f"rb{bi}_{h}")
                    nc.vector.reciprocal(rb[:, 0:nqb], mix[:, h, 0:nqb, 64])
                    nc.vector.tensor_tensor(
                        mixn[:, 0:nqb, h, :], mix[:, h, 0:nqb, 0:64],
                        rb[:, 0:nqb, None].to_broadcast((P, nqb, 64)), MULT)
                return mixn

            def outproj_stage1(bi, qb, mixn, tail=False):
                # transpose through the DMA xbar: no psum slot, no PE, no copy
                mt = smallp.tile([P, P], F16, tag="mt", name=f"mt{bi}_{qb}")
                eng = (nc.sync, nc.scalar)[qb % 2] if tail else nc.sync
                eng.dma_start_transpose(
                    mt, mixn[:, qb].rearrange("p h c -> p (h c)"))
                return mt

            def outproj_stage2(bq0, qb, mt, tag):
                po = ring_tile([P, D], f"po{tag}_{qb}")
                nc.tensor.matmul(po, lhsT=mt, rhs=wo_sb, start=True, stop=True)
                ob = smallp.tile([P, D], F32, tag="ob", name=f"ob{tag}_{qb}")
                nc.vector.tensor_copy(ob, po)
                r0 = bq0 + qb * P
                if qb % 2 == 0:
                    nc.gpsimd.dma_start(out[r0:r0 + P, :], ob)
                else:
                    nc.sync.dma_start(out[r0:r0 + P, :], ob)

            # =================== emission ===================
            proj_q(0, nc.sync.dma_start)
            proj_q(1, nc.gpsimd.dma_start)
            proj_k(0, nc.sync.dma_start)
            proj_v(0, nc.gpsimd.dma_start)

            # q blocks: (q0, n_qb). The last block is small so the final
            # (unhidden) out-projection tail is half-size.
            QBLOCKS = [(0, 8), (1024, 8), (2048, 8), (3072, 4), (3584, 4)]
            prev = None       # (q0, nqb, mixn) of the finished block
            prev_mt = {}
            for bi, (bq0, nqb) in enumerate(QBLOCKS):
                for j in range(32):
                    if bi == 0:
                        g, r = j // 4, j % 4
                        if r == 0 and g + 1 < 8:
                            proj_k(g + 1, nc.sync.dma_start)
                            proj_v(g + 1, nc.gpsimd.dma_start)
                        if r == 2 and 1 <= g <= 3:
                            proj_q(2 * g, nc.sync.dma_start)
                            proj_q(2 * g + 1, nc.gpsimd.dma_start)
                    elif prev is not None:
                        # spread previous block's out-projection: transposes
                        # (DMA xbar, no ring) at j=1..nqb; out-proj matmuls
                        # in parity-preserving ring pairs from j=12
                        pq0, pnqb, pmixn = prev
                        if 1 <= j <= pnqb:
                            prev_mt[j - 1] = outproj_stage1(bi - 1, j - 1,
                                                            pmixn)
                        elif j >= 12 and (j - 12) % 3 == 0                                 and (j - 12) // 3 < pnqb // 2:
                            i = (j - 12) // 3
                            outproj_stage2(pq0, 2 * i, prev_mt.pop(2 * i),
                                           bi - 1)
                            outproj_stage2(pq0, 2 * i + 1,
                                           prev_mt.pop(2 * i + 1), bi - 1)
                    for h in range(2):
                        for qh in range(nqb // 4):
                            emit_s_exp(bq0, bi == 0, j, h, qh)
                            flush_pv(keep=12)
                flush_pv(keep=0)
                prev = (bq0, nqb, emit_outproj_head(bi, nqb))
            # tail: wave-ordered so each engine runs its ops back-to-back
            fq0, fnqb, fmixn = prev
            mts = [outproj_stage1(len(QBLOCKS) - 1, qb, fmixn, tail=True)
                   for qb in range(fnqb)]
            for qb in range(fnqb):
                outproj_stage2(fq0, qb, mts[qb], "tail")


    nc.compile()
    return nc


def get_nc(with_bias=False):
    if with_bias not in _NC:
        _NC[with_bias] = build(with_bias)
    return _NC[with_bias]


def make_in_maps(q, k, v, Wq, bq, Wk, bk, Wv, bv, Wo, bo, with_bias=False):
    q = np.asarray(q, np.float32)
    k = np.asarray(k, np.float32)
    v = np.asarray(v, np.float32)
    Wq = np.asarray(Wq, np.float16)
    Wk = np.asarray(Wk, np.float16)
    Wv = np.asarray(Wv, np.float16)
    Wo = np.asarray(Wo, np.float16)
    ident = np.eye(P, dtype=np.float32)
    xts = {}
    for n in range(2):
        xts[n] = (np.ascontiguousarray(q[n].T).astype(np.float16),
                  np.ascontiguousarray(k[n].T).astype(np.float16),
                  np.ascontiguousarray(v[n].T).astype(np.float16))
    in_maps = []
    for c in range(8):
        n, hp = c // 4, c % 4
        sl = slice(P * hp, P * (hp + 1))
        xq, xk, xv = xts[n]
        m = {
            "xqt": xq, "xkt": xk, "xvt": xv,
            "wq": np.ascontiguousarray(Wq[:, sl]),
            "wk": np.ascontiguousarray(Wk[:, sl]),
            "wv": np.ascontiguousarray(Wv[:, sl]),
            "wo": np.ascontiguousarray(Wo[sl, :]),
            "ident": ident,
        }
        if with_bias:
            m["bqr"] = np.asarray(bq, np.float16)[sl].reshape(1, P)
            m["bkr"] = np.asarray(bk, np.float16)[sl].reshape(1, P)
            m["bvr"] = np.asarray(bv, np.float16)[sl].reshape(1, P)
        in_maps.append(m)
    return in_maps


def assemble(results, bo):
    bo = np.asarray(bo, np.float32)
    out = np.zeros((2, L, D), np.float32)
    for c in range(8):
        out[c // 4] += results[c]["out"]
    out += bo[None, None, :]
    return out


def kernel(q, k, v, Wq, bq, Wk, bk, Wv, bv, Wo, bo):
    with_bias = any(
        np.any(np.asarray(b)) for b in (bq, bk, bv))
    nc = get_nc(with_bias)
    in_maps = make_in_maps(q, k, v, Wq, bq, Wk, bk, Wv, bv, Wo, bo,
                           with_bias=with_bias)
    res = bass_utils.run_bass_kernel_spmd(nc, in_maps, core_ids=list(range(8)))
    return assemble(res.results, bo)


if __name__ == "__main__":
    build()
    print("build ok")


# revision 41
# speedup vs baseline: 1.7050x; 1.0120x over previous
"""Multi-head attention (q/k/v projections + softmax attention + out-projection)
on 8 Trainium2 NeuronCores.

Sharding: 16 (batch, head) units over 8 cores -> core c handles batch n = c//4
and head pair hp = c%4 (channels 128*hp : 128*hp+128). Host sums the 4 partial
outputs per batch and adds bo.

v4 design (baseline was 418us):
  - fp16 operand universe: x, W, qpt/kpt [128ch, 4096] fp16, vp [128kv, 32,
    130] fp16 (cols 64/129 = ones for sumexp), Wo fp16, mixnT fp16.
  - S^T tiles [128 kv, 512 q] fp32 PSUM: 1 fp16 matmul each (kpt slice
    stationary, qpt slice moving). The 1/sqrt(64) scale folds into exp.
  - exp split across two engines: ACT tiles get exact exp -> fp16 est;
    DVE tiles get a Schraudolph fp16 bit-trick (tensor_scalar -> int16,
    bitcast to fp16; ~3% sawtooth err on ~45% of tiles -> ~1.4e-2 end err).
  - PV with est as the STATIONARY operand and a 65-wide moving vp slice
    (64 v-cols + ones): out[q, 0:65] lands in a 128-f32 slot of the mix
    accumulator [128, 2(h), 8(qb), 128] (4 banks; col 64 = sumexp). Each
    tile's 4 PV matmuls hit exactly one bank, so PSUM accumulation groups
    are started/stopped once per bank per lqc (start_tensor_calc clears
    has_written bank-wide).
  - normalization is per-partition (q on partitions): one reciprocal (mix
    col 64) + one tensor_tensor per 1024 q. PE transpose (fp32, identity)
    flips mixn[q,c] -> mixnT[c,q] into a ring psum slot, DVE copies to fp16
    SBUF, one fp16 matmul per 128-q block does the out-projection, DVE
    copies to SBUF, gpsimd DMAs out.
  - PSUM: unified 4-tag ring [128,512] (S tiles, projection psums, transpose
    slots, out-proj psums) + mix (4 banks) = 8 banks.
  - PV emission lags S/exp by 3 tiles so the in-order PE queue never waits
    on an exp in flight; projections for chunk g+1 are interleaved into the
    attention sweep of chunk g (lqc 0); out-projection of lqc L interleaves
    into the first attention steps of lqc L+1.
"""

import numpy as np

import concourse.bacc as bacc
import concourse.mybir as mybir
import concourse.tile as tile
from concourse import bass_utils

P = 128
L = 4096
D = 512
F32 = mybir.dt.float32
F16 = mybir.dt.float16
I16 = mybir.dt.int16
AF = mybir.ActivationFunctionType
MULT = mybir.AluOpType.mult
ADD = mybir.AluOpType.add

# Schraudolph fp16 bit-trick: i16 = A_S * s + B_S, bitcast int16->fp16
# approximates exp(0.125*s). C tuned for min max-rel-err (~3.0%).
A_S = 0.125 * 1477.3195458126642
B_S = 15360.0 - 56.0

_NC = {}


def _dve_tile(t, block0):
    """Which exp tiles take the DVE bit-trick path (rest go to ACT).

    Block 0 is PE-bound (projections) and ACT has spare capacity there, so
    it runs a 1/4 DVE split; later blocks run ~0.45 to balance ACT/DVE/PE.
    """
    if block0 and t < 64:
        return t % 4 == 1       # projections + DMA dominate: light DVE
    if block0:
        return (t * 5) % 12 < 5  # late block 0: relieve the ACT backlog
    return (t * 9) % 20 < 9


def build(with_bias=False):
    nc = bacc.Bacc("TRN2", target_bir_lowering=False, debug=False)

    xqt = nc.dram_tensor("xqt", (D, L), F16, kind="ExternalInput").ap()
    xkt = nc.dram_tensor("xkt", (D, L), F16, kind="ExternalInput").ap()
    xvt = nc.dram_tensor("xvt", (D, L), F16, kind="ExternalInput").ap()
    wq = nc.dram_tensor("wq", (D, P), F16, kind="ExternalInput").ap()
    wk = nc.dram_tensor("wk", (D, P), F16, kind="ExternalInput").ap()
    wv = nc.dram_tensor("wv", (D, P), F16, kind="ExternalInput").ap()
    wo = nc.dram_tensor("wo", (P, D), F16, kind="ExternalInput").ap()
    ident = nc.dram_tensor("ident", (P, P), F32, kind="ExternalInput").ap()
    if with_bias:
        bqr = nc.dram_tensor("bqr", (1, P), F16, kind="ExternalInput").ap()
        bkr = nc.dram_tensor("bkr", (1, P), F16, kind="ExternalInput").ap()
        bvr = nc.dram_tensor("bvr", (1, P), F16, kind="ExternalInput").ap()
    out = nc.dram_tensor("out", (L, D), F32, kind="ExternalOutput").ap()

    xqv = xqt.rearrange("(o p) l -> p o l", p=P)
    xkv = xkt.rearrange("(o p) l -> p o l", p=P)
    xvv = xvt.rearrange("(o p) l -> p o l", p=P)

    with tile.TileContext(nc) as tc:
        with tc.tile_pool(name="const", bufs=1) as const, \
             tc.tile_pool(name="persist", bufs=1) as persist, \
             tc.tile_pool(name="pps", bufs=1, space="PSUM") as pps, \
             tc.tile_pool(name="xs", bufs=2) as xs, \
             tc.tile_pool(name="ring", bufs=1, space="PSUM") as ring, \
             tc.tile_pool(name="estp", bufs=16) as estp, \
             tc.tile_pool(name="mixnp", bufs=2) as mixnp, \
             tc.tile_pool(name="smallp", bufs=8) as smallp:

            # ---- constants ----
            wq_sb = const.tile([P, 4, P], F16, tag="wq")
            nc.sync.dma_start(wq_sb, wq.rearrange("(o p) m -> p o m", p=P))
            wk_sb = const.tile([P, 4, P], F16, tag="wk")
            nc.sync.dma_start(wk_sb, wk.rearrange("(o p) m -> p o m", p=P))
            wv_sb = const.tile([P, 4, P], F16, tag="wv")
            nc.sync.dma_start(wv_sb, wv.rearrange("(o p) m -> p o m", p=P))
            wo_sb = const.tile([P, D], F16, tag="wo")
            nc.sync.dma_start(wo_sb, wo)
            id_sb = const.tile([P, P], F32, tag="id")
            nc.sync.dma_start(id_sb, ident)
            if with_bias:
                bq_sb = const.tile([1, P], F16, tag="bq")
                nc.sync.dma_start(bq_sb, bqr)
                bk_sb = const.tile([1, P], F16, tag="bk")
                nc.sync.dma_start(bk_sb, bkr)
                bv_sb = const.tile([1, P], F16, tag="bv")
                nc.sync.dma_start(bv_sb, bvr)
                ones_q = const.tile([1, D], F16, tag="onesq")
                nc.scalar.activation(
                    ones_q, bq_sb[:, 0:1].to_broadcast((1, D)),
                    AF.Identity, bias=1.0, scale=0.0)
                ones_kv = const.tile([1, P], F16, tag="oneskv")
                nc.scalar.activation(
                    ones_kv, bq_sb[:, 0:1].to_broadcast((1, P)),
                    AF.Identity, bias=1.0, scale=0.0)

            # ---- persistent tiles ----
            qpt = persist.tile([P, L], F16, tag="qpt")
            kpt = persist.tile([P, L], F16, tag="kpt")
            vp = persist.tile([P, 32, 130], F16, tag="vp")
            # ones columns (64 for h0 slice 0:65, 129 for h1 slice 65:130)
            nc.scalar.activation(
                vp[:, :, 64:130:65],
                wq_sb[:, 0:1, 0:1].to_broadcast((P, 32, 2)),
                AF.Identity, bias=1.0, scale=0.0)

            # mix[:, h, qb, 0:64] = sum_kv p*v ; mix[:, h, qb, 64] = sumexp
            mix = pps.tile([P, 2, 8, P], F32, tag="mix")      # 4 banks

            ring_n = [0]

            def ring_tile(shape, name):
                t = ring.tile(shape, F32, tag=f"r{ring_n[0] % 4}", name=name)
                ring_n[0] += 1
                return t

            # ---- projection emitters ----
            def proj_x(ch, q_engine, xv_, w_sb, dst, b_sb, xtag, pname):
                sl = slice(ch * D, (ch + 1) * D)
                xt = xs.tile([P, 4, D], F16, tag=xtag)
                q_engine(xt, xv_[:, :, sl])
                ps = ring_tile([P, D], pname)
                for dk in range(4):
                    nc.tensor.matmul(ps, lhsT=w_sb[:, dk, :], rhs=xt[:, dk, :],
                                     start=(dk == 0),
                                     stop=(dk == 3 and not with_bias))
                if with_bias:
                    nc.tensor.matmul(ps, lhsT=b_sb, rhs=ones_q,
                                     start=False, stop=True)
                # q/k converts feed the S matmuls directly (critical path):
                # keep them off the DVE exp queue, ACT has slack in block 0
                nc.scalar.activation(dst[:, sl], ps, AF.Identity)

            def proj_q(ch, q_engine):
                proj_x(ch, q_engine, xqv, wq_sb, qpt,
                       bq_sb if with_bias else None, "xq", f"pq{ch}")

            def proj_k(ch, q_engine):
                proj_x(ch, q_engine, xkv, wk_sb, kpt,
                       bk_sb if with_bias else None, "xk", f"pk{ch}")

            def proj_v(ch, q_engine):
                sl = slice(ch * D, (ch + 1) * D)
                xt = xs.tile([P, 4, D], F16, tag="xv")
                q_engine(xt, xvv[:, :, sl])
                # one ring tile; 4 sequential accumulation groups in the bank
                ps = ring_tile([P, 4, P], f"pv{ch}")
                for jb in range(4):
                    j = ch * 4 + jb
                    for dk in range(4):
                        nc.tensor.matmul(
                            ps[:, jb, :], lhsT=xt[:, dk, jb * P:(jb + 1) * P],
                            rhs=wv_sb[:, dk, :],
                            start=(dk == 0), stop=(dk == 3 and not with_bias))
                    if with_bias:
                        nc.tensor.matmul(ps[:, jb, :], lhsT=ones_kv, rhs=bv_sb,
                                         start=False, stop=True)
                    # write v channels: h0 -> cols 0:64, h1 -> cols 65:129
                    nc.vector.tensor_copy(_vp_dst(j), ps[:, jb, :])

            def _vp_dst(j):
                # [128, 2, 64] strided view: cols 0:64 and 65:129
                return vp[:, j, 0:130].rearrange("p (h c) -> p h c", c=65)[
                    :, :, 0:64]

            # ---- attention tile helpers ----
            pend = []   # emitted S/exp tiles whose PV is pending

            tcount = [0]

            def emit_s_exp(bq0, block0, j, h, qh):
                t = tcount[0]
                tcount[0] += 1
                st = ring_tile([P, D], f"st{t}")
                q0 = bq0 + qh * D
                nc.tensor.matmul(
                    st,
                    lhsT=kpt[64 * h:64 * h + 64, j * P:(j + 1) * P],
                    rhs=qpt[64 * h:64 * h + 64, q0:q0 + D],
                    start=True, stop=True)
                if _dve_tile(t, block0):
                    est = estp.tile([P, D], I16, tag="estD", name=f"eD{t}")
                    nc.vector.tensor_scalar(est, st, A_S, B_S, MULT, ADD)
                    est_f = est.bitcast(F16)
                else:
                    est = estp.tile([P, D], F16, tag="estA", name=f"eA{t}")
                    nc.scalar.activation(est, st, AF.Exp, scale=0.125)
                    est_f = est
                pend.append((j, h, qh, est_f))

            def emit_pv(j, h, qh, est_f):
                for i in range(4):
                    qb = qh * 4 + i
                    stat = est_f[:, i * P:(i + 1) * P]
                    nc.tensor.matmul(
                        mix[:, h, qb, 0:65], lhsT=stat,
                        rhs=vp[:, j, 65 * h:65 * h + 65],
                        start=(j == 0 and i == 0), stop=(j == 31 and i == 3))

            def flush_pv(keep):
                while len(pend) > keep:
                    emit_pv(*pend.pop(0))

            # ---- out-projection (for finished lqc) ----
            def emit_outproj_head(bi, nqb):
                # per-head recip+TT so each mix bank pair is released as
                # soon as its own normalize has read it; fp16 out so the
                # transpose can go through the DMA xbar (2-byte only)
                mixn = mixnp.tile([P, 8, 2, 64], F16, tag="mixn",
                                  name=f"mixn{bi}")
                for h in range(2):
                    rb = smallp.tile([P, 8], F32, tag=f"rb{h}",
                                     name=f"rb{bi}_{h}")
                    nc.vector.reciprocal(rb[:, 0:nqb], mix[:, h, 0:nqb, 64])
                    nc.vector.tensor_tensor(
                        mixn[:, 0:nqb, h, :], mix[:, h, 0:nqb, 0:64],
                        rb[:, 0:nqb, None].to_broadcast((P, nqb, 64)), MULT)
                return mixn

            def outproj_stage1(bi, qb, mixn, tail=False):
                # transpose through the DMA xbar: no psum slot, no PE, no copy
                mt = smallp.tile([P, P], F16, tag="mt", name=f"mt{bi}_{qb}")
                eng = (nc.sync, nc.scalar)[qb % 2] if tail else nc.sync
                eng.dma_start_transpose(
                    mt, mixn[:, qb].rearrange("p h c -> p (h c)"))
                return mt

            def outproj_stage2(bq0, qb, mt, tag):
                po = ring_tile([P, D], f"po{tag}_{qb}")
                nc.tensor.matmul(po, lhsT=mt, rhs=wo_sb, start=True, stop=True)
                ob = smallp.tile([P, D], F32, tag="ob", name=f"ob{tag}_{qb}")
                nc.vector.tensor_copy(ob, po)
                r0 = bq0 + qb * P
                if qb % 2 == 0:
                    nc.gpsimd.dma_start(out[r0:r0 + P, :], ob)
                else:
                    nc.sync.dma_start(out[r0:r0 + P, :], ob)

            # =================== emission ===================
            proj_q(0, nc.sync.dma_start)
            proj_q(1, nc.gpsimd.dma_start)
            proj_k(0, nc.sync.dma_start)
            proj_v(0, nc.gpsimd.dma_start)

            # q blocks: (q0, n_qb). The last block is small so the final
            # (unhidden) out-projection tail is half-size.
            QBLOCKS = [(0, 8), (1024, 8), (2048, 8), (3072, 4), (3584, 4)]
            prev = None       # (q0, nqb, mixn) of the finished block
            prev_mt = {}
            for bi, (bq0, nqb) in enumerate(QBLOCKS):
                for j in range(32):
                    if bi == 0:
                        g, r = j // 4, j % 4
                        if r == 0 and g + 1 < 8:
                            proj_k(g + 1, nc.sync.dma_start)
                            proj_v(g + 1, nc.gpsimd.dma_start)
                        if r == 2 and 1 <= g <= 3:
                            proj_q(2 * g, nc.sync.dma_start)
                            proj_q(2 * g + 1, nc.gpsimd.dma_start)
                    elif prev is not None:
                        # spread previous block's out-projection: transposes
                        # (DMA xbar, no ring) at j=1..nqb; out-proj matmuls
                        # in parity-preserving ring pairs from j=12
                        pq0, pnqb, pmixn = prev
                        if 1 <= j <= pnqb:
                            prev_mt[j - 1] = outproj_stage1(bi - 1, j - 1,
                                                            pmixn)
                        elif j >= 12 and (j - 12) % 3 == 0                                 and (j - 12) // 3 < pnqb // 2:
                            i = (j - 12) // 3
                            outproj_stage2(pq0, 2 * i, prev_mt.pop(2 * i),
                                           bi - 1)
                            outproj_stage2(pq0, 2 * i + 1,
                                           prev_mt.pop(2 * i + 1), bi - 1)
                    for h in range(2):
                        for qh in range(nqb // 4):
                            emit_s_exp(bq0, bi == 0, j, h, qh)
                            flush_pv(keep=12)
                flush_pv(keep=0)
                prev = (bq0, nqb, emit_outproj_head(bi, nqb))
            # tail: wave-ordered so each engine runs its ops back-to-back
            fq0, fnqb, fmixn = prev
            mts = [outproj_stage1(len(QBLOCKS) - 1, qb, fmixn, tail=True)
                   for qb in range(fnqb)]
            for qb in range(fnqb):
                outproj_stage2(fq0, qb, mts[qb], "tail")


    nc.compile()
    return nc


def get_nc(with_bias=False):
    if with_bias not in _NC:
        _NC[with_bias] = build(with_bias)
    return _NC[with_bias]


def make_in_maps(q, k, v, Wq, bq, Wk, bk, Wv, bv, Wo, bo, with_bias=False):
    q = np.asarray(q, np.float32)
    k = np.asarray(k, np.float32)
    v = np.asarray(v, np.float32)
    Wq = np.asarray(Wq, np.float16)
    Wk = np.asarray(Wk, np.float16)
    Wv = np.asarray(Wv, np.float16)
    Wo = np.asarray(Wo, np.float16)
    ident = np.eye(P, dtype=np.float32)
    xts = {}
    for n in range(2):
        xts[n] = (np.ascontiguousarray(q[n].T).astype(np.float16),
                  np.ascontiguousarray(k[n].T).astype(np.float16),
                  np.ascontiguousarray(v[n].T).astype(np.float16))
    in_maps = []
    for c in range(8):
        n, hp = c // 4, c % 4
        sl = slice(P * hp, P * (hp + 1))
        xq, xk, xv = xts[n]
        m = {
            "xqt": xq, "xkt": xk, "xvt": xv,
            "wq": np.ascontiguousarray(Wq[:, sl]),
            "wk": np.ascontiguousarray(Wk[:, sl]),
            "wv": np.ascontiguousarray(Wv[:, sl]),
            "wo": np.ascontiguousarray(Wo[sl, :]),
            "ident": ident,
        }
        if with_bias:
            m["bqr"] = np.asarray(bq, np.float16)[sl].reshape(1, P)
            m["bkr"] = np.asarray(bk, np.float16)[sl].reshape(1, P)
            m["bvr"] = np.asarray(bv, np.float16)[sl].reshape(1, P)
        in_maps.append(m)
    return in_maps


def assemble(results, bo):
    bo = np.asarray(bo, np.float32)
    out = np.zeros((2, L, D), np.float32)
    for c in range(8):
        out[c // 4] += results[c]["out"]
    out += bo[None, None, :]
    return out


def kernel(q, k, v, Wq, bq, Wk, bk, Wv, bv, Wo, bo):
    with_bias = any(
        np.any(np.asarray(b)) for b in (bq, bk, bv))
    nc = get_nc(with_bias)
    in_maps = make_in_maps(q, k, v, Wq, bq, Wk, bk, Wv, bv, Wo, bo,
                           with_bias=with_bias)
    res = bass_utils.run_bass_kernel_spmd(nc, in_maps, core_ids=list(range(8)))
    return assemble(res.results, bo)


if __name__ == "__main__":
    build()
    print("build ok")


# revision 48
# speedup vs baseline: 1.7247x; 1.0116x over previous
"""Multi-head attention (q/k/v projections + softmax attention + out-projection)
on 8 Trainium2 NeuronCores.

Sharding: 16 (batch, head) units over 8 cores -> core c handles batch n = c//4
and head pair hp = c%4 (channels 128*hp : 128*hp+128). Host sums the 4 partial
outputs per batch and adds bo.

v4 design (baseline was 418us):
  - fp16 operand universe: x, W, qpt/kpt [128ch, 4096] fp16, vp [128kv, 32,
    130] fp16 (cols 64/129 = ones for sumexp), Wo fp16, mixnT fp16.
  - S^T tiles [128 kv, 512 q] fp32 PSUM: 1 fp16 matmul each (kpt slice
    stationary, qpt slice moving). The 1/sqrt(64) scale folds into exp.
  - exp split across two engines: ACT tiles get exact exp -> fp16 est;
    DVE tiles get a Schraudolph fp16 bit-trick (tensor_scalar -> int16,
    bitcast to fp16; ~3% sawtooth err on ~45% of tiles -> ~1.4e-2 end err).
  - PV with est as the STATIONARY operand and a 65-wide moving vp slice
    (64 v-cols + ones): out[q, 0:65] lands in a 128-f32 slot of the mix
    accumulator [128, 2(h), 8(qb), 128] (4 banks; col 64 = sumexp). Each
    tile's 4 PV matmuls hit exactly one bank, so PSUM accumulation groups
    are started/stopped once per bank per lqc (start_tensor_calc clears
    has_written bank-wide).
  - normalization is per-partition (q on partitions): one reciprocal (mix
    col 64) + one tensor_tensor per 1024 q. PE transpose (fp32, identity)
    flips mixn[q,c] -> mixnT[c,q] into a ring psum slot, DVE copies to fp16
    SBUF, one fp16 matmul per 128-q block does the out-projection, DVE
    copies to SBUF, gpsimd DMAs out.
  - PSUM: unified 4-tag ring [128,512] (S tiles, projection psums, transpose
    slots, out-proj psums) + mix (4 banks) = 8 banks.
  - PV emission lags S/exp by 3 tiles so the in-order PE queue never waits
    on an exp in flight; projections for chunk g+1 are interleaved into the
    attention sweep of chunk g (lqc 0); out-projection of lqc L interleaves
    into the first attention steps of lqc L+1.
"""

import numpy as np

import concourse.bacc as bacc
import concourse.mybir as mybir
import concourse.tile as tile
from concourse import bass_utils

P = 128
L = 4096
D = 512
F32 = mybir.dt.float32
F16 = mybir.dt.float16
I16 = mybir.dt.int16
AF = mybir.ActivationFunctionType
MULT = mybir.AluOpType.mult
ADD = mybir.AluOpType.add

# Schraudolph fp16 bit-trick: i16 = A_S * s + B_S, bitcast int16->fp16
# approximates exp(0.125*s). C tuned for min max-rel-err (~3.0%).
A_S = 0.125 * 1477.3195458126642
B_S = 15360.0 - 56.0

_NC = {}


def _dve_tile(t, block0):
    """Which exp tiles take the DVE bit-trick path (rest go to ACT).

    Block 0 is PE-bound (projections) and ACT has spare capacity there, so
    it runs a 1/4 DVE split; later blocks run ~0.45 to balance ACT/DVE/PE.
    """
    if block0 and t < 64:
        return t % 4 == 1       # projections + DMA dominate: light DVE
    if block0:
        return (t * 5) % 12 < 5  # late block 0: relieve the ACT backlog
    return (t * 9) % 20 < 9


def build(with_bias=False):
    nc = bacc.Bacc("TRN2", target_bir_lowering=False, debug=False)

    xqt = nc.dram_tensor("xqt", (D, L), F16, kind="ExternalInput").ap()
    xkt = nc.dram_tensor("xkt", (D, L), F16, kind="ExternalInput").ap()
    xvt = nc.dram_tensor("xvt", (D, L), F16, kind="ExternalInput").ap()
    wq = nc.dram_tensor("wq", (D, P), F16, kind="ExternalInput").ap()
    wk = nc.dram_tensor("wk", (D, P), F16, kind="ExternalInput").ap()
    wv = nc.dram_tensor("wv", (D, P), F16, kind="ExternalInput").ap()
    wo = nc.dram_tensor("wo", (P, D), F16, kind="ExternalInput").ap()
    ident = nc.dram_tensor("ident", (P, P), F32, kind="ExternalInput").ap()
    if with_bias:
        bqr = nc.dram_tensor("bqr", (1, P), F16, kind="ExternalInput").ap()
        bkr = nc.dram_tensor("bkr", (1, P), F16, kind="ExternalInput").ap()
        bvr = nc.dram_tensor("bvr", (1, P), F16, kind="ExternalInput").ap()
    out = nc.dram_tensor("out", (L, D), F32, kind="ExternalOutput").ap()

    xqv = xqt.rearrange("(o p) l -> p o l", p=P)
    xkv = xkt.rearrange("(o p) l -> p o l", p=P)
    xvv = xvt.rearrange("(o p) l -> p o l", p=P)

    with tile.TileContext(nc) as tc:
        with tc.tile_pool(name="const", bufs=1) as const, \
             tc.tile_pool(name="persist", bufs=1) as persist, \
             tc.tile_pool(name="pps", bufs=1, space="PSUM") as pps, \
             tc.tile_pool(name="xs", bufs=2) as xs, \
             tc.tile_pool(name="ring", bufs=1, space="PSUM") as ring, \
             tc.tile_pool(name="estp", bufs=16) as estp, \
             tc.tile_pool(name="mixnp", bufs=2) as mixnp, \
             tc.tile_pool(name="smallp", bufs=8) as smallp:

            # ---- constants ----
            wq_sb = const.tile([P, 4, P], F16, tag="wq")
            nc.sync.dma_start(wq_sb, wq.rearrange("(o p) m -> p o m", p=P))
            wk_sb = const.tile([P, 4, P], F16, tag="wk")
            nc.sync.dma_start(wk_sb, wk.rearrange("(o p) m -> p o m", p=P))
            wv_sb = const.tile([P, 4, P], F16, tag="wv")
            nc.sync.dma_start(wv_sb, wv.rearrange("(o p) m -> p o m", p=P))
            wo_sb = const.tile([P, D], F16, tag="wo")
            nc.sync.dma_start(wo_sb, wo)
            id_sb = const.tile([P, P], F32, tag="id")
            nc.sync.dma_start(id_sb, ident)
            if with_bias:
                bq_sb = const.tile([1, P], F16, tag="bq")
                nc.sync.dma_start(bq_sb, bqr)
                bk_sb = const.tile([1, P], F16, tag="bk")
                nc.sync.dma_start(bk_sb, bkr)
                bv_sb = const.tile([1, P], F16, tag="bv")
                nc.sync.dma_start(bv_sb, bvr)
                ones_q = const.tile([1, D], F16, tag="onesq")
                nc.scalar.activation(
                    ones_q, bq_sb[:, 0:1].to_broadcast((1, D)),
                    AF.Identity, bias=1.0, scale=0.0)
                ones_kv = const.tile([1, P], F16, tag="oneskv")
                nc.scalar.activation(
                    ones_kv, bq_sb[:, 0:1].to_broadcast((1, P)),
                    AF.Identity, bias=1.0, scale=0.0)

            # ---- persistent tiles ----
            qpt = persist.tile([P, L], F16, tag="qpt")
            kpt = persist.tile([P, L], F16, tag="kpt")
            vp = persist.tile([P, 32, 130], F16, tag="vp")
            # ones columns (64 for h0 slice 0:65, 129 for h1 slice 65:130)
            nc.scalar.activation(
                vp[:, :, 64:130:65],
                wq_sb[:, 0:1, 0:1].to_broadcast((P, 32, 2)),
                AF.Identity, bias=1.0, scale=0.0)

            # mix[:, h, qb, 0:64] = sum_kv p*v ; mix[:, h, qb, 64] = sumexp
            mix = pps.tile([P, 2, 8, P], F32, tag="mix")      # 4 banks

            ring_n = [0]

            def ring_tile(shape, name):
                t = ring.tile(shape, F32, tag=f"r{ring_n[0] % 4}", name=name)
                ring_n[0] += 1
                return t

            # ---- projection emitters ----
            def proj_x(ch, q_engine, xv_, w_sb, dst, b_sb, xtag, pname):
                sl = slice(ch * D, (ch + 1) * D)
                xt = xs.tile([P, 4, D], F16, tag=xtag)
                q_engine(xt, xv_[:, :, sl])
                ps = ring_tile([P, D], pname)
                for dk in range(4):
                    nc.tensor.matmul(ps, lhsT=w_sb[:, dk, :], rhs=xt[:, dk, :],
                                     start=(dk == 0),
                                     stop=(dk == 3 and not with_bias))
                if with_bias:
                    nc.tensor.matmul(ps, lhsT=b_sb, rhs=ones_q,
                                     start=False, stop=True)
                # q/k converts feed the S matmuls directly (critical path):
                # keep them off the DVE exp queue, ACT has slack in block 0
                nc.scalar.activation(dst[:, sl], ps, AF.Identity)

            def proj_q(ch, q_engine):
                proj_x(ch, q_engine, xqv, wq_sb, qpt,
                       bq_sb if with_bias else None, "xq", f"pq{ch}")

            def proj_k(ch, q_engine):
                proj_x(ch, q_engine, xkv, wk_sb, kpt,
                       bk_sb if with_bias else None, "xk", f"pk{ch}")

            def proj_v(ch, q_engine):
                sl = slice(ch * D, (ch + 1) * D)
                xt = xs.tile([P, 4, D], F16, tag="xv")
                q_engine(xt, xvv[:, :, sl])
                # one ring tile; 4 sequential accumulation groups in the bank
                ps = ring_tile([P, 4, P], f"pv{ch}")
                for jb in range(4):
                    j = ch * 4 + jb
                    for dk in range(4):
                        nc.tensor.matmul(
                            ps[:, jb, :], lhsT=xt[:, dk, jb * P:(jb + 1) * P],
                            rhs=wv_sb[:, dk, :],
                            start=(dk == 0), stop=(dk == 3 and not with_bias))
                    if with_bias:
                        nc.tensor.matmul(ps[:, jb, :], lhsT=ones_kv, rhs=bv_sb,
                                         start=False, stop=True)
                    # write v channels: h0 -> cols 0:64, h1 -> cols 65:129
                    nc.vector.tensor_copy(_vp_dst(j), ps[:, jb, :])

            def _vp_dst(j):
                # [128, 2, 64] strided view: cols 0:64 and 65:129
                return vp[:, j, 0:130].rearrange("p (h c) -> p h c", c=65)[
                    :, :, 0:64]

            # ---- attention tile helpers ----
            pend = []   # emitted S/exp tiles whose PV is pending

            tcount = [0]

            def emit_s_exp(bq0, block0, j, h, qh):
                t = tcount[0]
                tcount[0] += 1
                st = ring_tile([P, D], f"st{t}")
                q0 = bq0 + qh * D
                nc.tensor.matmul(
                    st,
                    lhsT=kpt[64 * h:64 * h + 64, j * P:(j + 1) * P],
                    rhs=qpt[64 * h:64 * h + 64, q0:q0 + D],
                    start=True, stop=True)
                if _dve_tile(t, block0):
                    est = estp.tile([P, D], I16, tag="estD", name=f"eD{t}")
                    nc.vector.tensor_scalar(est, st, A_S, B_S, MULT, ADD)
                    est_f = est.bitcast(F16)
                else:
                    est = estp.tile([P, D], F16, tag="estA", name=f"eA{t}")
                    nc.scalar.activation(est, st, AF.Exp, scale=0.125)
                    est_f = est
                pend.append((j, h, qh, est_f))

            def emit_pv(j, h, qh, est_f):
                for i in range(4):
                    qb = qh * 4 + i
                    stat = est_f[:, i * P:(i + 1) * P]
                    nc.tensor.matmul(
                        mix[:, h, qb, 0:65], lhsT=stat,
                        rhs=vp[:, j, 65 * h:65 * h + 65],
                        start=(j == 0 and i == 0), stop=(j == 31 and i == 3))

            def flush_pv(keep):
                while len(pend) > keep:
                    emit_pv(*pend.pop(0))

            # ---- out-projection (for finished lqc) ----
            def emit_outproj_head(bi, nqb):
                # per-head recip+TT so each mix bank pair is released as
                # soon as its own normalize has read it; fp16 out so the
                # transpose can go through the DMA xbar (2-byte only)
                mixn = mixnp.tile([P, 8, 2, 64], F16, tag="mixn",
                                  name=f"mixn{bi}")
                for h in range(2):
                    rb = smallp.tile([P, 8], F32, tag=f"rb{h}",
                                     name=f"rb{bi}_{h}")
                    nc.vector.reciprocal(rb[:, 0:nqb], mix[:, h, 0:nqb, 64])
                    nc.vector.tensor_tensor(
                        mixn[:, 0:nqb, h, :], mix[:, h, 0:nqb, 0:64],
                        rb[:, 0:nqb, None].to_broadcast((P, nqb, 64)), MULT)
                return mixn

            def outproj_stage1(bi, qb, mixn, tail=False):
                # transpose through the DMA xbar: no psum slot, no PE, no copy
                mt = smallp.tile([P, P], F16, tag="mt", name=f"mt{bi}_{qb}")
                eng = (nc.sync, nc.scalar)[qb % 2] if tail else nc.sync
                eng.dma_start_transpose(
                    mt, mixn[:, qb].rearrange("p h c -> p (h c)"))
                return mt

            def outproj_stage2(bq0, qb, mt, tag):
                po = ring_tile([P, D], f"po{tag}_{qb}")
                nc.tensor.matmul(po, lhsT=mt, rhs=wo_sb, start=True, stop=True)
                ob = smallp.tile([P, D], F32, tag="ob", name=f"ob{tag}_{qb}")
                nc.vector.tensor_copy(ob, po)
                r0 = bq0 + qb * P
                if qb % 2 == 0:
                    nc.gpsimd.dma_start(out[r0:r0 + P, :], ob)
                else:
                    nc.sync.dma_start(out[r0:r0 + P, :], ob)

            # =================== emission ===================
            proj_q(0, nc.sync.dma_start)
            proj_q(1, nc.gpsimd.dma_start)
            proj_k(0, nc.sync.dma_start)
            proj_v(0, nc.gpsimd.dma_start)

            # q blocks: (q0, n_qb). The last block is small so the final
            # (unhidden) out-projection tail is half-size.
            QBLOCKS = [(0, 8), (1024, 8), (2048, 8), (3072, 4), (3584, 4)]
            prev = None       # (q0, nqb, mixn) of the finished block
            prev_mt = {}
            for bi, (bq0, nqb) in enumerate(QBLOCKS):
                for j in range(32):
                    if bi == 0:
                        g, r = j // 4, j % 4
                        if r == 0 and g + 1 < 8:
                            proj_k(g + 1, nc.sync.dma_start)
                            proj_v(g + 1, nc.gpsimd.dma_start)
                        if r == 2 and 1 <= g <= 3:
                            proj_q(2 * g, nc.sync.dma_start)
                            proj_q(2 * g + 1, nc.gpsimd.dma_start)
                    elif prev is not None:
                        # spread previous block's out-projection: transposes
                        # (DMA xbar, no ring) at j=1..nqb; out-proj matmuls
                        # in parity-preserving ring pairs from j=12
                        pq0, pnqb, pmixn = prev
                        if 1 <= j <= pnqb:
                            prev_mt[j - 1] = outproj_stage1(bi - 1, j - 1,
                                                            pmixn)
                        elif j >= 12 and (j - 12) % 5 == 0                                 and (j - 12) // 5 < pnqb // 2:
                            i = (j - 12) // 5
                            outproj_stage2(pq0, 2 * i, prev_mt.pop(2 * i),
                                           bi - 1)
                            outproj_stage2(pq0, 2 * i + 1,
                                           prev_mt.pop(2 * i + 1), bi - 1)
                    for h in range(2):
                        for qh in range(nqb // 4):
                            emit_s_exp(bq0, bi == 0, j, h, qh)
                            flush_pv(keep=12)
                flush_pv(keep=0)
                prev = (bq0, nqb, emit_outproj_head(bi, nqb))
            # tail: wave-ordered so each engine runs its ops back-to-back
            fq0, fnqb, fmixn = prev
            mts = [outproj_stage1(len(QBLOCKS) - 1, qb, fmixn, tail=True)
                   for qb in range(fnqb)]
            for qb in range(fnqb):
                outproj_stage2(fq0, qb, mts[qb], "tail")


    nc.compile()
    return nc


def get_nc(with_bias=False):
    if with_bias not in _NC:
        _NC[with_bias] = build(with_bias)
    return _NC[with_bias]


def make_in_maps(q, k, v, Wq, bq, Wk, bk, Wv, bv, Wo, bo, with_bias=False):
    q = np.asarray(q, np.float32)
    k = np.asarray(k, np.float32)
    v = np.asarray(v, np.float32)
    Wq = np.asarray(Wq, np.float16)
    Wk = np.asarray(Wk, np.float16)
    Wv = np.asarray(Wv, np.float16)
    Wo = np.asarray(Wo, np.float16)
    ident = np.eye(P, dtype=np.float32)
    xts = {}
    for n in range(2):
        xts[n] = (np.ascontiguousarray(q[n].T).astype(np.float16),
                  np.ascontiguousarray(k[n].T).astype(np.float16),
                  np.ascontiguousarray(v[n].T).astype(np.float16))
    in_maps = []
    for c in range(8):
        n, hp = c // 4, c % 4
        sl = slice(P * hp, P * (hp + 1))
        xq, xk, xv = xts[n]
        m = {
            "xqt": xq, "xkt": xk, "xvt": xv,
            "wq": np.ascontiguousarray(Wq[:, sl]),
            "wk": np.ascontiguousarray(Wk[:, sl]),
            "wv": np.ascontiguousarray(Wv[:, sl]),
            "wo": np.ascontiguousarray(Wo[sl, :]),
            "ident": ident,
        }
        if with_bias:
            m["bqr"] = np.asarray(bq, np.float16)[sl].reshape(1, P)
            m["bkr"] = np.asarray(bk, np.float16)[sl].reshape(1, P)
            m["bvr"] = np.asarray(bv, np.float16)[sl].reshape(1, P)
        in_maps.append(m)
    return in_maps


def assemble(results, bo):
    bo = np.asarray(bo, np.float32)
    out = np.zeros((2, L, D), np.float32)
    for c in range(8):
        out[c // 4] += results[c]["out"]
    out += bo[None, None, :]
    return out


def kernel(q, k, v, Wq, bq, Wk, bk, Wv, bv, Wo, bo):
    with_bias = any(
        np.any(np.asarray(b)) for b in (bq, bk, bv))
    nc = get_nc(with_bias)
    in_maps = make_in_maps(q, k, v, Wq, bq, Wk, bk, Wv, bv, Wo, bo,
                           with_bias=with_bias)
    res = bass_utils.run_bass_kernel_spmd(nc, in_maps, core_ids=list(range(8)))
    return assemble(res.results, bo)


if __name__ == "__main__":
    build()
    print("build ok")
